# revision 1
# baseline (speedup 1.0000x reference)
"""Trainium2 Bass kernel for nn_HandIntersectionLoss.

Strategy
--------
Pure data parallel over batch: 64 batches -> 8 cores x 8 local batches.

The reference math is reformulated so the tensor engine does the heavy
per-(point, face) lifting via K=5 matmuls (polynomial expansion of the
Van Oosterom / Strackee solid-angle terms):

    |A-p|^2          = |A|^2 - 2 p.A + |p|^2
    (A-p).(B-p)      = A.B - p.(A+B) + |p|^2
    det(A-p,B-p,C-p) = A.(BxC) - p.(AxB + BxC + CxA)

With moving rows [-2px,-2py,-2pz, 1, |p|^2] a single matmul against
per-face constant columns produces la^2, lb^2, lc^2, ab, bc, ca, det
for a [128 points x 500 faces] block.  The per-element chain
(denominator assembly + range-reduced atan2) runs on DVE/ACT:

    atan2(det, den) = 2*atan(det / (rho + |den|))            (den >= 0)
                    = sign(det)*pi - 2*atan(det/(rho+|den|)) (den < 0)
    rho = sqrt(det^2 + den^2 + 1e-20)   -> |atan input| <= 1 always

inside(p) <=> sum_f atan2 > pi <=> sum_f half > pi/2.  Min-distance
uses the same matmul trick + free-dim min-reduce.

Scalar-engine table sets force a two-pass structure (sqrt and arctan
live in different ACT table sets): pass A computes through tt=det/dd
(sqrt set), pass B does the arctan + quadrant correction (sigmoid set),
with den/tt staged in SBUF between passes (super-groups of 16 blocks to
fit the SBUF column budget).

Host side does only index gathers / constant prep (O(B*F)) - all
O(B*P*F) math runs on device.
"""
import os
import sys
import numpy as np

sys.path.insert(0, '/opt/trn_rl_repo')

B, V_FULL, V_HAND, V_LOOP, N_FACES = 64, 6890, 250, 20, 500
P = V_HAND + 1          # 251 points/verts per hand (incl. lid)
PPAD = 256
NCORES = 8
NB = B // NCORES        # local batches per core
NBD = NB * 2            # (batch, dir) pairs per core
NBLK = NBD * 2          # blocks per core: x2 point-chunks of 128
SUPER = 16              # blocks per two-pass super-group
F = N_FACES
HALF_PI = float(np.pi / 2)

_compiled = None        # cached compiled program across kernel() calls
last_exec_time_ns = None


# --------------------------------------------------------------------------
# host prep: index gathers + per-face constants (float64 -> float32 round)
# --------------------------------------------------------------------------

def _host_prep(inputs):
    verts = np.asarray(inputs['verts_batch'], dtype=np.float32)
    idx = {k: np.asarray(inputs[k], dtype=np.int64) for k in (
        'hand_verts_inds_left', 'hand_verts_inds_right',
        'hand_loop_verts_inds_left', 'hand_loop_verts_inds_right',
        'hand_faces_left', 'hand_faces_right')}

    pts = {}
    for d, (hi, li) in enumerate([
            ('hand_verts_inds_left', 'hand_loop_verts_inds_left'),
            ('hand_verts_inds_right', 'hand_loop_verts_inds_right')]):
        h = verts[:, idx[hi]]                                   # [B,250,3]
        lid = verts[:, idx[li]].mean(axis=1, keepdims=True, dtype=np.float32)
        pts[d] = np.concatenate([h, lid], axis=1)               # [B,251,3] f32

    faces = {0: idx['hand_faces_left'], 1: idx['hand_faces_right']}

    lhsT = np.zeros((B, 2, 5, PPAD), np.float32)
    frhs = np.zeros((B, 2, 5, 7, 512), np.float32)   # [.., K-row, group, face]
    mrhs = np.zeros((B, 2, 5, PPAD), np.float32)

    for d in range(2):
        p = pts[d].astype(np.float64)
        pad = np.full((B, PPAD - P, 3), 1e3)
        pf = np.concatenate([p, pad], axis=1)                   # [B,256,3]
        lhsT[:, d, 0:3] = (-2.0 * pf.transpose(0, 2, 1)).astype(np.float32)
        lhsT[:, d, 3] = 1.0
        lhsT[:, d, 4] = (pf ** 2).sum(-1).astype(np.float32)

        ov = pts[1 - d].astype(np.float64)                      # other-hand verts
        tri = ov[:, faces[1 - d]]                               # [B,500,3,3]
        A, Bv, C = tri[:, :, 0], tri[:, :, 1], tri[:, :, 2]
        n = np.cross(A, Bv) + np.cross(Bv, C) + np.cross(C, A)
        d0 = np.einsum('bfi,bfi->bf', A, np.cross(Bv, C))
        groups = [
            (A,            (A ** 2).sum(-1),                1.0),
            (Bv,           (Bv ** 2).sum(-1),               1.0),
            (C,            (C ** 2).sum(-1),                1.0),
            ((A + Bv) / 2, np.einsum('bfi,bfi->bf', A, Bv), 1.0),
            ((Bv + C) / 2, np.einsum('bfi,bfi->bf', Bv, C), 1.0),
            ((C + A) / 2,  np.einsum('bfi,bfi->bf', C, A),  1.0),
            (n / 2,        d0,                              0.0),
        ]
        for g, (xyz, c3, ones) in enumerate(groups):
            frhs[:, d, 0:3, g, :F] = xyz.transpose(0, 2, 1).astype(np.float32)
            frhs[:, d, 3, g, :F] = c3.astype(np.float32)
            frhs[:, d, 4, g, :F] = ones

        mrhs[:, d, 0:3, :P] = ov.transpose(0, 2, 1).astype(np.float32)
        mrhs[:, d, 3, :P] = (ov ** 2).sum(-1).astype(np.float32)
        mrhs[:, d, 4, :P] = 1.0

    return lhsT, frhs, mrhs


# --------------------------------------------------------------------------
# device kernel
# --------------------------------------------------------------------------

def _kernel_body(tc, lhsT_d, frhs_d, mrhs_d, loss_d, dbg=None):
    import concourse.mybir as mybir
    nc = tc.nc
    fp32 = mybir.dt.float32
    AF = mybir.ActivationFunctionType
    OP = mybir.AluOpType
    AX = mybir.AxisListType.X

    with (
        tc.tile_pool(name="const", bufs=1) as cpool,
        tc.tile_pool(name="store", bufs=1) as spool,
        tc.tile_pool(name="stage", bufs=2) as stpool,
        tc.tile_pool(name="iface", bufs=2) as ipool,
        tc.tile_pool(name="dve", bufs=1) as vpool,
    ):
        lhsT_sb = cpool.tile([5, NBD, PPAD], fp32)
        nc.sync.dma_start(lhsT_sb[:], lhsT_d[:])

        ones = cpool.tile([128, 1], fp32)
        nc.vector.memset(ones[:], 1.0)

        sacc = cpool.tile([128, NBLK], fp32)     # per block: sum_f half-angle
        minda = cpool.tile([128, NBLK], fp32)    # per block: clamped min d^2
        denoms = spool.tile([128, SUPER, 512], fp32)
        tts = spool.tile([128, SUPER, 512], fp32)

        def pass_a(ppool, i, j):
            bd, ch = divmod(i, 2)
            if ch == 0:
                fstage = stpool.tile([5, 7, 512], fp32, tag="fstage")
                mstage = stpool.tile([5, PPAD], fp32, tag="mstage")
                nc.sync.dma_start(fstage[:], frhs_d[:, bd])
                nc.sync.dma_start(mstage[:], mrhs_d[:, bd])
                pass_a.stage = (fstage, mstage)
            fstage, mstage = pass_a.stage
            lhs = lhsT_sb[:, bd, ch * 128:(ch + 1) * 128]       # [5,128]

            wind = ppool.tile([128, 7, 512], fp32, tag="wind")
            md = ppool.tile([128, 256], fp32, tag="md")

            for g in range(7):
                nc.tensor.matmul(wind[:, g, :F], lhs, fstage[:, g, :F])
            nc.tensor.matmul(md[:, :P], lhs, mstage[:, :P])

            # min-distance: free-dim min, clamp at 0 (matmul roundoff)
            mind = vpool.tile([128, 1], fp32, tag="mind")
            nc.vector.tensor_reduce(mind[:], md[:, :P], AX, OP.min)
            nc.vector.tensor_scalar(minda[:, i:i + 1], mind[:], 0.0, None,
                                    OP.max)

            # norms: clamp squared lengths at 0 (fp32 matmul roundoff), sqrt
            rl = ipool.tile([128, 3, 512], fp32, tag="rl")
            for g in range(3):
                nc.scalar.activation(rl[:, g, :F], wind[:, g, :F], AF.Relu)
            la = ipool.tile([128, 512], fp32, tag="la")
            lb = ipool.tile([128, 512], fp32, tag="lb")
            lc = ipool.tile([128, 512], fp32, tag="lc")
            nc.scalar.activation(la[:, :F], rl[:, 0, :F], AF.Sqrt)
            nc.scalar.activation(lb[:, :F], rl[:, 1, :F], AF.Sqrt)
            nc.scalar.activation(lc[:, :F], rl[:, 2, :F], AF.Sqrt)
            dets = ipool.tile([128, 512], fp32, tag="dets")
            nc.scalar.activation(dets[:, :F], wind[:, 6, :F], AF.Copy)

            # denominator chain (DVE); PSUM reads scheduled early
            u = vpool.tile([128, 512], fp32, tag="u")
            r4 = vpool.tile([128, 512], fp32, tag="r4")
            s5 = vpool.tile([128, 512], fp32, tag="s5")
            v = vpool.tile([128, 512], fp32, tag="v")
            w = vpool.tile([128, 512], fp32, tag="w")
            t6 = vpool.tile([128, 512], fp32, tag="t6")
            nc.vector.tensor_tensor(r4[:, :F], wind[:, 4, :F], la[:, :F],
                                    OP.mult)
            nc.vector.tensor_tensor(s5[:, :F], wind[:, 5, :F], lb[:, :F],
                                    OP.mult)
            nc.vector.tensor_tensor(u[:, :F], la[:, :F], lb[:, :F], OP.mult)
            nc.vector.tensor_tensor(v[:, :F], u[:, :F], wind[:, 3, :F],
                                    OP.add)

            # rest of the chain is SBUF-only
            w_ = w[:, :F]
            nc.vector.tensor_tensor(w_, v[:, :F], lc[:, :F], OP.mult)
            nc.vector.tensor_tensor(t6[:, :F], r4[:, :F], s5[:, :F], OP.add)
            den = denoms[:, j, :F]
            nc.vector.tensor_tensor(den, w_, t6[:, :F], OP.add)

            # half-angle atan2 range reduction: tt = det / (rho + |den|)
            xx = ipool.tile([128, 512], fp32, tag="xx")
            yy = ipool.tile([128, 512], fp32, tag="yy")
            ss = vpool.tile([128, 512], fp32, tag="ss", bufs=2)
            rho = ipool.tile([128, 512], fp32, tag="rho")
            axd = ipool.tile([128, 512], fp32, tag="axd")
            dd = vpool.tile([128, 512], fp32, tag="dd")
            rd = vpool.tile([128, 512], fp32, tag="rd")
            nc.scalar.activation(xx[:, :F], den, AF.Square)
            nc.scalar.activation(yy[:, :F], dets[:, :F], AF.Square)
            nc.vector.scalar_tensor_tensor(ss[:, :F], xx[:, :F], 1e-20,
                                           yy[:, :F], OP.add, OP.add)
            nc.scalar.activation(rho[:, :F], ss[:, :F], AF.Sqrt)
            nc.scalar.activation(axd[:, :F], den, AF.Abs)
            nc.vector.tensor_tensor(dd[:, :F], rho[:, :F], axd[:, :F], OP.add)
            nc.vector.reciprocal_approx_fast(rd[:, :F], dd[:, :F])
            nc.vector.tensor_tensor(tts[:, j, :F], dets[:, :F], rd[:, :F],
                                    OP.mult)
            if dbg is not None and i == 0:
                wcopy = vpool.tile([128, 7, 512], fp32, tag="wcopy")
                for g in range(7):
                    nc.scalar.activation(wcopy[:, g, :F], wind[:, g, :F], AF.Copy)
                    nc.sync.dma_start(dbg["wind0"][:, g, :F], wcopy[:, g, :F])
                nc.sync.dma_start(dbg["den0"][:, :F], denoms[:, 0, :F])
                nc.sync.dma_start(dbg["tt0"][:, :F], tts[:, 0, :F])

        def pass_b(i, j):
            den = denoms[:, j, :F]
            tt = tts[:, j, :F]
            sgn = ipool.tile([128, 512], fp32, tag="sgn")
            spi = ipool.tile([128, 512], fp32, tag="spi")
            atn = ipool.tile([128, 512], fp32, tag="atn")
            c0 = vpool.tile([128, 512], fp32, tag="c0")
            c1 = vpool.tile([128, 512], fp32, tag="c1")
            sd = vpool.tile([128, 512], fp32, tag="sd")
            nc.scalar.activation(sgn[:, :F], tt, AF.Sign)
            nc.scalar.mul(spi[:, :F], sgn[:, :F], HALF_PI)
            nc.scalar.activation(atn[:, :F], tt, AF.Arctan)
            # half = atn + [den<0]*(pi/2*sign(det) - 2*atn); sign(det)==sign(atn)
            # (gpsimd offload of these was tried: fails in the bass2jax/PJRT
            # lowering, so they stay on DVE)
            nc.vector.scalar_tensor_tensor(c0[:, :F], atn[:, :F], -2.0,
                                           spi[:, :F], OP.mult, OP.add)
            nc.vector.scalar_tensor_tensor(c1[:, :F], den, 0.0,
                                           c0[:, :F], OP.is_lt, OP.mult)
            nc.vector.scalar_tensor_tensor(sd[:, :F], atn[:, :F], 0.0,
                                           c1[:, :F], OP.add, OP.add,
                                           accum_out=sacc[:, i:i + 1])

        with tc.tile_pool(name="psum", bufs=1, space="PSUM") as ppool:
            for s in range(NBLK // SUPER):
                for j in range(SUPER):
                    pass_a(ppool, s * SUPER + j, j)
                tc.no_sync_barrier()
                for j in range(SUPER):
                    pass_b(s * SUPER + j, j)
                tc.no_sync_barrier()

        # ---------------- final: depth * inside, partition-reduce ----------
        inside = cpool.tile([128, NBLK], fp32)
        depth = cpool.tile([128, NBLK], fp32)
        contrib = cpool.tile([128, NBLK], fp32)
        beps = cpool.tile([128, 1], fp32)
        nc.vector.memset(beps[:], 1e-12)
        nc.vector.tensor_scalar(inside[:], sacc[:], HALF_PI, None, OP.is_gt)
        nc.scalar.activation(depth[:], minda[:], AF.Sqrt, bias=beps[:])
        nc.vector.tensor_tensor(contrib[:], depth[:], inside[:], OP.mult)

        with tc.tile_pool(name="psum2", bufs=1, space="PSUM") as p2:
            lpsum = p2.tile([NBLK, 1], fp32)
            nc.tensor.matmul(lpsum[:], contrib[:], ones[:])
            loss_sb = cpool.tile([NBLK, 1], fp32)
            nc.scalar.activation(loss_sb[:], lpsum[:], AF.Copy)
            nc.sync.dma_start(loss_d[:], loss_sb[:])
        if dbg is not None:
            nc.sync.dma_start(dbg["sacc"][:], sacc[:])
            nc.sync.dma_start(dbg["minda"][:], minda[:])


def _build():
    global _compiled
    if _compiled is not None:
        return _compiled
    import concourse.bacc as bacc
    import concourse.mybir as mybir
    import concourse.tile as tile

    nc = bacc.Bacc("TRN2", target_bir_lowering=False, debug=False,
                   num_devices=NCORES)
    fp32 = mybir.dt.float32
    lhsT_d = nc.dram_tensor("lhsT", (5, NBD, PPAD), fp32, kind="ExternalInput").ap()
    frhs_d = nc.dram_tensor("frhs", (5, NBD, 7, 512), fp32, kind="ExternalInput").ap()
    mrhs_d = nc.dram_tensor("mrhs", (5, NBD, PPAD), fp32, kind="ExternalInput").ap()
    loss_d = nc.dram_tensor("loss", (NBLK, 1), fp32, kind="ExternalOutput").ap()

    with tile.TileContext(nc) as tc:
        _kernel_body(tc, lhsT_d, frhs_d, mrhs_d, loss_d)
    nc.compile()
    _compiled = nc
    return nc


# --------------------------------------------------------------------------
# entry point
# --------------------------------------------------------------------------

def _in_maps(lhsT, frhs, mrhs):
    maps = []
    for c in range(NCORES):
        bs = slice(c * NB, (c + 1) * NB)
        maps.append({
            "lhsT": lhsT[bs].reshape(NBD, 5, PPAD).transpose(1, 0, 2).copy(),
            "frhs": frhs[bs].reshape(NBD, 5, 7, 512).transpose(1, 0, 2, 3).copy(),
            "mrhs": mrhs[bs].reshape(NBD, 5, PPAD).transpose(1, 0, 2).copy(),
        })
    return maps


def kernel(**inputs) -> np.ndarray:
    global last_exec_time_ns
    from concourse.bass_utils import run_bass_kernel_spmd

    lhsT, frhs, mrhs = _host_prep(inputs)
    nc = _build()

    trace = bool(int(os.environ.get("HAND_KERNEL_TRACE", "0")))
    res = run_bass_kernel_spmd(nc, _in_maps(lhsT, frhs, mrhs),
                               list(range(NCORES)), trace=trace)
    last_exec_time_ns = res.exec_time_ns

    loss = np.zeros(B, np.float32)
    for c in range(NCORES):
        out = np.asarray(res.results[c]["loss"], np.float32).reshape(NBLK)
        # block i = (b_loc*2 + dir)*2 + chunk
        loss[c * NB:(c + 1) * NB] = out.reshape(NB, 4).sum(axis=1)
    return loss



# revision 2
# speedup vs baseline: 2.1363x; 2.1363x over previous
"""Trainium2 Bass kernel for nn_HandIntersectionLoss.

Strategy
--------
Pure data parallel over batch: 64 batches -> 8 cores x 8 local batches.

The reference math is reformulated so the tensor engine does the heavy
per-(point, face) lifting via K=5 matmuls (polynomial expansion of the
Van Oosterom / Strackee solid-angle terms):

    |A-p|^2          = |A|^2 - 2 p.A + |p|^2
    (A-p).(B-p)      = A.B - p.(A+B) + |p|^2
    det(A-p,B-p,C-p) = A.(BxC) - p.(AxB + BxC + CxA)

With moving rows [-2px,-2py,-2pz, 1, |p|^2] a single matmul against
per-face constant columns produces la^2, lb^2, lc^2, ab, bc, ca, det
for a [128 points x 500 faces] block.  The per-element chain
(denominator assembly + range-reduced atan2) runs on DVE/ACT:

    atan2(det, den) = 2*atan(det / (rho + |den|))            (den >= 0)
                    = sign(det)*pi - 2*atan(det/(rho+|den|)) (den < 0)
    rho = sqrt(det^2 + den^2 + 1e-20)   -> |atan input| <= 1 always

inside(p) <=> sum_f atan2 > pi <=> sum_f half > pi/2.  Min-distance
uses the same matmul trick + free-dim min-reduce.

Scalar-engine table sets force a two-pass structure (sqrt and arctan
live in different ACT table sets): pass A computes through tt=det/dd
(sqrt set), pass B does the arctan + quadrant correction (sigmoid set),
with den/tt staged in SBUF between passes (super-groups of 16 blocks to
fit the SBUF column budget).

Host side does only index gathers / constant prep (O(B*F)) - all
O(B*P*F) math runs on device.
"""
import os
import sys
import numpy as np

sys.path.insert(0, '/opt/trn_rl_repo')

B, V_FULL, V_HAND, V_LOOP, N_FACES = 64, 6890, 250, 20, 500
P = V_HAND + 1          # 251 points/verts per hand (incl. lid)
PPAD = 256
NCORES = 8
NB = B // NCORES        # local batches per core
NBD = NB * 2            # (batch, dir) pairs per core
NBLK = NBD * 2          # blocks per core: x2 point-chunks of 128
SUPER = 16              # blocks per two-pass super-group
F = N_FACES
HALF_PI = float(np.pi / 2)

_compiled = None        # cached compiled program across kernel() calls
last_exec_time_ns = None


# --------------------------------------------------------------------------
# host prep: index gathers + per-face constants (float64 -> float32 round)
# --------------------------------------------------------------------------

def _host_prep(inputs):
    verts = np.asarray(inputs['verts_batch'], dtype=np.float32)
    idx = {k: np.asarray(inputs[k], dtype=np.int64) for k in (
        'hand_verts_inds_left', 'hand_verts_inds_right',
        'hand_loop_verts_inds_left', 'hand_loop_verts_inds_right',
        'hand_faces_left', 'hand_faces_right')}

    pts = {}
    for d, (hi, li) in enumerate([
            ('hand_verts_inds_left', 'hand_loop_verts_inds_left'),
            ('hand_verts_inds_right', 'hand_loop_verts_inds_right')]):
        h = verts[:, idx[hi]]                                   # [B,250,3]
        lid = verts[:, idx[li]].mean(axis=1, keepdims=True, dtype=np.float32)
        pts[d] = np.concatenate([h, lid], axis=1)               # [B,251,3] f32

    faces = {0: idx['hand_faces_left'], 1: idx['hand_faces_right']}

    lhsT = np.zeros((B, 2, 5, PPAD), np.float32)
    frhs = np.zeros((B, 2, 5, 7, 512), np.float32)   # [.., K-row, group, face]
    mrhs = np.zeros((B, 2, 5, PPAD), np.float32)

    for d in range(2):
        p = pts[d].astype(np.float64)
        pad = np.full((B, PPAD - P, 3), 1e3)
        pf = np.concatenate([p, pad], axis=1)                   # [B,256,3]
        lhsT[:, d, 0:3] = (-2.0 * pf.transpose(0, 2, 1)).astype(np.float32)
        lhsT[:, d, 3] = 1.0
        lhsT[:, d, 4] = (pf ** 2).sum(-1).astype(np.float32)

        ov = pts[1 - d].astype(np.float64)                      # other-hand verts
        tri = ov[:, faces[1 - d]]                               # [B,500,3,3]
        A, Bv, C = tri[:, :, 0], tri[:, :, 1], tri[:, :, 2]
        n = np.cross(A, Bv) + np.cross(Bv, C) + np.cross(C, A)
        d0 = np.einsum('bfi,bfi->bf', A, np.cross(Bv, C))
        groups = [
            (A,            (A ** 2).sum(-1),                1.0),
            (Bv,           (Bv ** 2).sum(-1),               1.0),
            (C,            (C ** 2).sum(-1),                1.0),
            ((A + Bv) / 2, np.einsum('bfi,bfi->bf', A, Bv), 1.0),
            ((Bv + C) / 2, np.einsum('bfi,bfi->bf', Bv, C), 1.0),
            ((C + A) / 2,  np.einsum('bfi,bfi->bf', C, A),  1.0),
            (n / 2,        d0,                              0.0),
        ]
        for g, (xyz, c3, ones) in enumerate(groups):
            frhs[:, d, 0:3, g, :F] = xyz.transpose(0, 2, 1).astype(np.float32)
            frhs[:, d, 3, g, :F] = c3.astype(np.float32)
            frhs[:, d, 4, g, :F] = ones

        mrhs[:, d, 0:3, :P] = ov.transpose(0, 2, 1).astype(np.float32)
        mrhs[:, d, 3, :P] = (ov ** 2).sum(-1).astype(np.float32)
        mrhs[:, d, 4, :P] = 1.0

    return lhsT, frhs, mrhs


# --------------------------------------------------------------------------
# device kernel
# --------------------------------------------------------------------------

def _kernel_body(tc, lhsT_d, frhs_d, mrhs_d, loss_d, dbg=None):
    import concourse.mybir as mybir
    nc = tc.nc
    fp32 = mybir.dt.float32
    AF = mybir.ActivationFunctionType
    OP = mybir.AluOpType
    AX = mybir.AxisListType.X

    with (
        tc.tile_pool(name="const", bufs=1) as cpool,
        tc.tile_pool(name="store", bufs=1) as spool,
        tc.tile_pool(name="stage", bufs=2) as stpool,
        tc.tile_pool(name="iface", bufs=2) as ipool,
        tc.tile_pool(name="dve", bufs=1) as vpool,
    ):
        lhsT_sb = cpool.tile([5, NBD, PPAD], fp32)
        nc.sync.dma_start(lhsT_sb[:], lhsT_d[:])

        ones = cpool.tile([128, 1], fp32)
        nc.vector.memset(ones[:], 1.0)

        sacc = cpool.tile([128, NBLK], fp32)     # per block: sum_f half-angle
        minda = cpool.tile([128, NBLK], fp32)    # per block: clamped min d^2
        denoms = spool.tile([128, SUPER, 512], fp32)
        tts = spool.tile([128, SUPER, 512], fp32)

        def pass_a(ppool, i, j):
            bd, ch = divmod(i, 2)
            if ch == 0:
                fstage = stpool.tile([5, 7, 512], fp32, tag="fstage")
                mstage = stpool.tile([5, PPAD], fp32, tag="mstage")
                nc.sync.dma_start(fstage[:], frhs_d[:, bd])
                nc.sync.dma_start(mstage[:], mrhs_d[:, bd])
                pass_a.stage = (fstage, mstage)
            fstage, mstage = pass_a.stage
            lhs = lhsT_sb[:, bd, ch * 128:(ch + 1) * 128]       # [5,128]

            wind = ppool.tile([128, 7, 512], fp32, tag="wind")
            md = ppool.tile([128, 256], fp32, tag="md")

            for g in range(7):
                nc.tensor.matmul(wind[:, g, :F], lhs, fstage[:, g, :F])
            nc.tensor.matmul(md[:, :P], lhs, mstage[:, :P])

            # min-distance: free-dim min, clamp at 0 (matmul roundoff)
            mind = vpool.tile([128, 1], fp32, tag="mind")
            nc.vector.tensor_reduce(mind[:], md[:, :P], AX, OP.min)
            nc.vector.tensor_scalar(minda[:, i:i + 1], mind[:], 0.0, None,
                                    OP.max)

            # norms: clamp squared lengths at 0 (fp32 matmul roundoff), sqrt
            rl = ipool.tile([128, 3, 512], fp32, tag="rl")
            for g in range(3):
                nc.scalar.activation(rl[:, g, :F], wind[:, g, :F], AF.Relu)
            la = ipool.tile([128, 512], fp32, tag="la")
            lb = ipool.tile([128, 512], fp32, tag="lb")
            lc = ipool.tile([128, 512], fp32, tag="lc")
            nc.scalar.activation(la[:, :F], rl[:, 0, :F], AF.Sqrt)
            nc.scalar.activation(lb[:, :F], rl[:, 1, :F], AF.Sqrt)
            nc.scalar.activation(lc[:, :F], rl[:, 2, :F], AF.Sqrt)
            dets = ipool.tile([128, 512], fp32, tag="dets")
            nc.scalar.activation(dets[:, :F], wind[:, 6, :F], AF.Copy)

            # denominator chain (DVE); PSUM reads scheduled early
            u = vpool.tile([128, 512], fp32, tag="u")
            r4 = vpool.tile([128, 512], fp32, tag="r4")
            s5 = vpool.tile([128, 512], fp32, tag="s5")
            v = vpool.tile([128, 512], fp32, tag="v")
            w = vpool.tile([128, 512], fp32, tag="w")
            t6 = vpool.tile([128, 512], fp32, tag="t6")
            nc.vector.tensor_tensor(r4[:, :F], wind[:, 4, :F], la[:, :F],
                                    OP.mult)
            nc.vector.tensor_tensor(s5[:, :F], wind[:, 5, :F], lb[:, :F],
                                    OP.mult)
            nc.vector.tensor_tensor(u[:, :F], la[:, :F], lb[:, :F], OP.mult)
            nc.vector.tensor_tensor(v[:, :F], u[:, :F], wind[:, 3, :F],
                                    OP.add)

            # rest of the chain is SBUF-only
            w_ = w[:, :F]
            nc.vector.tensor_tensor(w_, v[:, :F], lc[:, :F], OP.mult)
            nc.vector.tensor_tensor(t6[:, :F], r4[:, :F], s5[:, :F], OP.add)
            den = denoms[:, j, :F]
            nc.vector.tensor_tensor(den, w_, t6[:, :F], OP.add)

            # half-angle atan2 range reduction: tt = det / (rho + |den|)
            xx = ipool.tile([128, 512], fp32, tag="xx")
            yy = ipool.tile([128, 512], fp32, tag="yy")
            ss = vpool.tile([128, 512], fp32, tag="ss", bufs=2)
            rho = ipool.tile([128, 512], fp32, tag="rho")
            axd = ipool.tile([128, 512], fp32, tag="axd")
            dd = vpool.tile([128, 512], fp32, tag="dd")
            rd = vpool.tile([128, 512], fp32, tag="rd")
            nc.scalar.activation(xx[:, :F], den, AF.Square)
            nc.scalar.activation(yy[:, :F], dets[:, :F], AF.Square)
            nc.vector.scalar_tensor_tensor(ss[:, :F], xx[:, :F], 1e-20,
                                           yy[:, :F], OP.add, OP.add)
            nc.scalar.activation(rho[:, :F], ss[:, :F], AF.Sqrt)
            nc.scalar.activation(axd[:, :F], den, AF.Abs)
            nc.vector.tensor_tensor(dd[:, :F], rho[:, :F], axd[:, :F], OP.add)
            nc.vector.reciprocal_approx_fast(rd[:, :F], dd[:, :F])
            nc.vector.tensor_tensor(tts[:, j, :F], dets[:, :F], rd[:, :F],
                                    OP.mult)
            if dbg is not None and i == 0:
                wcopy = vpool.tile([128, 7, 512], fp32, tag="wcopy")
                for g in range(7):
                    nc.scalar.activation(wcopy[:, g, :F], wind[:, g, :F], AF.Copy)
                    nc.sync.dma_start(dbg["wind0"][:, g, :F], wcopy[:, g, :F])
                nc.sync.dma_start(dbg["den0"][:, :F], denoms[:, 0, :F])
                nc.sync.dma_start(dbg["tt0"][:, :F], tts[:, 0, :F])

        def pass_b(i, j):
            den = denoms[:, j, :F]
            tt = tts[:, j, :F]
            sgn = ipool.tile([128, 512], fp32, tag="sgn")
            spi = ipool.tile([128, 512], fp32, tag="spi")
            atn = ipool.tile([128, 512], fp32, tag="atn")
            c0 = vpool.tile([128, 512], fp32, tag="c0")
            c1 = vpool.tile([128, 512], fp32, tag="c1")
            sd = vpool.tile([128, 512], fp32, tag="sd")
            nc.scalar.activation(sgn[:, :F], tt, AF.Sign)
            nc.scalar.mul(spi[:, :F], sgn[:, :F], HALF_PI)
            nc.scalar.activation(atn[:, :F], tt, AF.Arctan)
            # half = atn + [den<0]*(pi/2*sign(det) - 2*atn); sign(det)==sign(atn)
            # (gpsimd offload of these was tried: fails in the bass2jax/PJRT
            # lowering, so they stay on DVE)
            nc.vector.scalar_tensor_tensor(c0[:, :F], atn[:, :F], -2.0,
                                           spi[:, :F], OP.mult, OP.add)
            nc.vector.scalar_tensor_tensor(c1[:, :F], den, 0.0,
                                           c0[:, :F], OP.is_lt, OP.mult)
            nc.vector.scalar_tensor_tensor(sd[:, :F], atn[:, :F], 0.0,
                                           c1[:, :F], OP.add, OP.add,
                                           accum_out=sacc[:, i:i + 1])

        with tc.tile_pool(name="psum", bufs=1, space="PSUM") as ppool:
            for s in range(NBLK // SUPER):
                for j in range(SUPER):
                    pass_a(ppool, s * SUPER + j, j)
                tc.no_sync_barrier()
                for j in range(SUPER):
                    pass_b(s * SUPER + j, j)
                tc.no_sync_barrier()

        # ---------------- final: depth * inside, partition-reduce ----------
        inside = cpool.tile([128, NBLK], fp32)
        depth = cpool.tile([128, NBLK], fp32)
        contrib = cpool.tile([128, NBLK], fp32)
        beps = cpool.tile([128, 1], fp32)
        nc.vector.memset(beps[:], 1e-12)
        nc.vector.tensor_scalar(inside[:], sacc[:], HALF_PI, None, OP.is_gt)
        nc.scalar.activation(depth[:], minda[:], AF.Sqrt, bias=beps[:])
        nc.vector.tensor_tensor(contrib[:], depth[:], inside[:], OP.mult)

        with tc.tile_pool(name="psum2", bufs=1, space="PSUM") as p2:
            lpsum = p2.tile([NBLK, 1], fp32)
            nc.tensor.matmul(lpsum[:], contrib[:], ones[:])
            loss_sb = cpool.tile([NBLK, 1], fp32)
            nc.scalar.activation(loss_sb[:], lpsum[:], AF.Copy)
            nc.sync.dma_start(loss_d[:], loss_sb[:])
        if dbg is not None:
            nc.sync.dma_start(dbg["sacc"][:], sacc[:])
            nc.sync.dma_start(dbg["minda"][:], minda[:])


def _build():
    global _compiled
    if _compiled is not None:
        return _compiled
    import concourse.bacc as bacc
    import concourse.mybir as mybir
    import concourse.tile as tile

    nc = bacc.Bacc("TRN2", target_bir_lowering=False, debug=False,
                   num_devices=NCORES)
    fp32 = mybir.dt.float32
    lhsT_d = nc.dram_tensor("lhsT", (5, NBD, PPAD), fp32, kind="ExternalInput").ap()
    frhs_d = nc.dram_tensor("frhs", (5, NBD, 7, 512), fp32, kind="ExternalInput").ap()
    mrhs_d = nc.dram_tensor("mrhs", (5, NBD, PPAD), fp32, kind="ExternalInput").ap()
    loss_d = nc.dram_tensor("loss", (NBLK, 1), fp32, kind="ExternalOutput").ap()

    with tile.TileContext(nc) as tc:
        _kernel_body(tc, lhsT_d, frhs_d, mrhs_d, loss_d)
    nc.compile()
    _compiled = nc
    return nc


# --------------------------------------------------------------------------
# entry point
# --------------------------------------------------------------------------

def _in_maps(lhsT, frhs, mrhs):
    maps = []
    for c in range(NCORES):
        bs = slice(c * NB, (c + 1) * NB)
        maps.append({
            "lhsT": lhsT[bs].reshape(NBD, 5, PPAD).transpose(1, 0, 2).copy(),
            "frhs": frhs[bs].reshape(NBD, 5, 7, 512).transpose(1, 0, 2, 3).copy(),
            "mrhs": mrhs[bs].reshape(NBD, 5, PPAD).transpose(1, 0, 2).copy(),
        })
    return maps


_runner = None          # cached jitted shard_map callable across kernel() calls


def _build_runner():
    """Jit the bass program once; reuse the compiled executable per call.

    Replicates run_bass_via_pjrt but caches the jitted callable, so repeat
    kernel() calls skip the jax retrace + XLA recompile (~150ms/call).
    """
    global _runner
    if _runner is not None:
        return _runner
    import jax
    from jax.sharding import Mesh, PartitionSpec
    from jax.experimental.shard_map import shard_map
    import concourse.mybir as mybir
    from concourse.bass2jax import (_bass_exec_p, partition_id_tensor,
                                    install_neuronx_cc_hook)

    nc = _build()
    install_neuronx_cc_hook()
    pname = nc.partition_id_tensor.name if nc.partition_id_tensor else None
    in_names, out_names, out_avals, zero_outs = [], [], [], []
    for alloc in nc.m.functions[0].allocations:
        if not isinstance(alloc, mybir.MemoryLocationSet):
            continue
        name = alloc.memorylocations[0].name
        if alloc.kind == "ExternalInput":
            if name != pname:
                in_names.append(name)
        elif alloc.kind == "ExternalOutput":
            out_names.append(name)
            shape = tuple(alloc.tensor_shape)
            dtype = mybir.dt.np(alloc.dtype)
            out_avals.append(jax.core.ShapedArray(shape, dtype))
            zero_outs.append(np.zeros(shape, dtype))
    n_params, n_outs = len(in_names), len(out_avals)
    in_names_full = in_names + out_names + ([pname] if pname else [])

    def _body(*args):
        operands = list(args)
        if pname is not None:
            operands.append(partition_id_tensor())
        return tuple(_bass_exec_p.bind(
            *operands, out_avals=tuple(out_avals), in_names=tuple(in_names_full),
            out_names=tuple(out_names), lowering_input_output_aliases=(),
            sim_require_finite=True, sim_require_nnan=True, nc=nc))

    devices = jax.devices()[:NCORES]
    mesh = Mesh(np.asarray(devices), ("core",))
    in_specs = (PartitionSpec("core"),) * (n_params + n_outs)
    out_specs = (PartitionSpec("core"),) * len(out_names)
    sharded = jax.jit(
        shard_map(_body, mesh=mesh, in_specs=in_specs, out_specs=out_specs,
                  check_rep=False),
        donate_argnums=tuple(range(n_params, n_params + n_outs)),
        keep_unused=True)
    czero_shapes = [((NCORES * z.shape[0],) + z.shape[1:], z.dtype)
                    for z in zero_outs]
    _runner = (sharded, in_names, czero_shapes)
    return _runner


def kernel(**inputs) -> np.ndarray:
    global last_exec_time_ns
    lhsT, frhs, mrhs = _host_prep(inputs)
    sharded, in_names, czero_shapes = _build_runner()
    maps = _in_maps(lhsT, frhs, mrhs)
    concat_in = [np.concatenate([maps[c][nm] for c in range(NCORES)], axis=0)
                 for nm in in_names]
    zeros = [np.zeros(s, d) for s, d in czero_shapes]
    out = sharded(*concat_in, *zeros)
    last_exec_time_ns = None

    o0 = np.asarray(out[0]).reshape(NCORES, NBLK)
    loss = np.zeros(B, np.float32)
    for c in range(NCORES):
        # block i = (b_loc*2 + dir)*2 + chunk
        loss[c * NB:(c + 1) * NB] = o0[c].reshape(NB, 4).sum(axis=1)
    return loss



# revision 5
# speedup vs baseline: 6.4676x; 3.0275x over previous
"""Trainium2 Bass kernel for nn_HandIntersectionLoss.

Strategy
--------
Pure data parallel over batch: 64 batches -> 8 cores x 8 local batches.

Wall-clock per call is dominated by the axon tunnel, so the host ships
only the gathered hand points (~140KB/core) and the device derives all
per-(batch,face) matmul constants itself:

  phase 0 (device):
    - one-hot face matrices from f32 face indices (K=1 broadcast matmul
      + is_equal against shipped iota columns)
    - triangle corners A,B,C per (batch,dir) via 2-chunk accumulated
      gather matmuls:  corners[3,500] = pts[128,3]^T @ onehot[128,500]
    - edges E1=B-A, E2=C-A; normal n = E1 x E2 via permutation-matmul
      rotations (engines cannot read partition offsets != 0)
    - dots |A|^2,.., 2A.B,.., 2A.n via ones/twos-column reduce matmuls
    - constants assembled into a persistent `staged` SBUF tile
      ([65,7,512]: 4 rows per (batch,dir) + shared coefficient row)
      via SBUF->SBUF DMAs (the only legal cross-partition mover)

  phase 1 (device): the proven compute loop. Per 128-point block:
    K=5 matmuls against staged constants produce la^2,lb^2,lc^2,
    2ab,2bc,2ca, 2det for [128 points x 500 faces]; per-element chain
    (denominator + range-reduced atan2) on DVE/ACT:

      atan2(det, den) = 2*atan(det / (rho + |den|))            (den >= 0)
                      = sign(det)*pi - 2*atan(det/(rho+|den|)) (den < 0)
      rho = sqrt(det^2 + den^2 + 1e-20)

    inside(p) <=> sum_f half > pi/2.  Min-distance via the same matmul
    trick against derived vert constants (mrhs) + free-dim min-reduce.
    Scalar-engine table sets force the two-pass structure (sqrt vs
    arctan live in different ACT table sets), staged in super-groups.

The jitted shard_map callable is cached across kernel() calls so repeat
calls skip jax retrace/XLA recompile entirely.

Group semantics (raw, no halving on device):
  g0..2: xyz=A|B|C,       c3=|A|^2..,  w=1
  g3..5: xyz=(A+B)..raw,  c3=2A.B..,   w=2   -> col = 2*(A-p).(B-p)
  g6:    xyz=n raw,       c3=2*A.n,    w=0   -> col = 2*det
pass_a compensates with x0.5 folded into existing scalar_tensor_tensor.
"""
import os
import sys
import numpy as np

sys.path.insert(0, '/opt/trn_rl_repo')

B, V_FULL, V_HAND, V_LOOP, N_FACES = 64, 6890, 250, 20, 500
P = V_HAND + 1          # 251 points/verts per hand (incl. lid)
PPAD = 256
NCORES = 8
NB = B // NCORES        # local batches per core
NBD = NB * 2            # (batch, dir) pairs per core
NBLK = NBD * 2          # blocks per core: x2 point-chunks of 128
SUPER = 8               # blocks per two-pass super-group
F = N_FACES
HALF_PI = float(np.pi / 2)

_compiled = None
_runner = None
last_exec_time_ns = None


# --------------------------------------------------------------------------
# host prep: index gathers only (all heavy constant math moved on-device)
# --------------------------------------------------------------------------

def _host_prep(inputs):
    verts = np.asarray(inputs['verts_batch'], dtype=np.float32)
    hi = [np.asarray(inputs['hand_verts_inds_left']),
          np.asarray(inputs['hand_verts_inds_right'])]
    li = [np.asarray(inputs['hand_loop_verts_inds_left']),
          np.asarray(inputs['hand_loop_verts_inds_right'])]
    fc = [np.asarray(inputs['hand_faces_left']),
          np.asarray(inputs['hand_faces_right'])]

    pts = np.full((B, 2, PPAD, 3), 1e3, np.float32)
    for d in range(2):
        pts[:, d, :V_HAND] = verts[:, hi[d]]
        pts[:, d, V_HAND] = verts[:, li[d]].mean(axis=1, dtype=np.float32)

    lhsT = np.empty((B, 2, 5, PPAD), np.float32)
    lhsT[:, :, 0:3] = -2.0 * pts.transpose(0, 1, 3, 2)
    lhsT[:, :, 3] = 1.0
    lhsT[:, :, 4] = (pts * pts).sum(-1)

    faces = np.full((1, 2, 3, 512), 300.0, np.float32)
    for s in range(2):
        faces[0, s, :, :F] = fc[s].T.astype(np.float32)

    cst = np.zeros((128, 8), np.float32)
    cst[:, 0] = np.arange(128, dtype=np.float32)
    cst[:, 1] = np.arange(128, 256, dtype=np.float32)
    for m in range(3):
        cst[(m + 1) % 3, 2 + m] = 1.0      # P1 (rot1)
        cst[(m + 2) % 3, 5 + m] = 1.0      # P2 (rot2)
    return lhsT, pts, faces, cst


def _in_maps(lhsT, pts, faces, cst):
    maps = []
    for c in range(NCORES):
        bs = slice(c * NB, (c + 1) * NB)
        lh = lhsT[bs].reshape(NBD, 5, PPAD).transpose(1, 0, 2)
        pt = pts[bs].reshape(NBD, 2, 128, 3).transpose(2, 1, 0, 3)
        maps.append({
            "lhsT": np.ascontiguousarray(lh),
            "pts": np.ascontiguousarray(pt),
            "faces": faces,
            "cst": cst,
        })
    return maps


# --------------------------------------------------------------------------
# device kernel
# --------------------------------------------------------------------------

def _kernel_body(tc, lhsT_d, pts_d, faces_d, cst_d, loss_d):
    import concourse.mybir as mybir
    nc = tc.nc
    fp32 = mybir.dt.float32
    AF = mybir.ActivationFunctionType
    OP = mybir.AluOpType
    AX = mybir.AxisListType.X

    with tc.tile_pool(name="const", bufs=1) as cpool:
        lhsT_sb = cpool.tile([5, NBD, PPAD], fp32)
        mrhs_sb = cpool.tile([5, NBD, PPAD], fp32)
        staged = cpool.tile([65, 7, 512], fp32)
        ones = cpool.tile([128, 1], fp32)
        beps = cpool.tile([128, 1], fp32)
        sacc = cpool.tile([128, NBLK], fp32)
        minda = cpool.tile([128, NBLK], fp32)
        nc.vector.memset(ones[:], 1.0)
        nc.vector.memset(beps[:], 1e-12)
        nc.sync.dma_start(lhsT_sb[:], lhsT_d[:])

        # ---------------- phase 0: derive constants on device ----------
        with tc.tile_pool(name="ph0", bufs=1) as zp:
            pts_sb = zp.tile([128, 2, NBD, 3], fp32)
            faces_sb = zp.tile([1, 2, 3, 512], fp32)
            cst_sb = zp.tile([128, 8], fp32)
            nc.sync.dma_start(pts_sb[:], pts_d[:])
            nc.sync.dma_start(faces_sb[:], faces_d[:])
            nc.sync.dma_start(cst_sb[:], cst_d[:])

            # mrhs: rows0..2 = -0.5*lhsT rows0..2 (= vert xyz),
            # row3 <- lhsT row4 (|v|^2), row4 <- lhsT row3 (ones)
            nc.vector.tensor_scalar(mrhs_sb[0:3], lhsT_sb[0:3], -0.5, None,
                                    OP.mult)
            nc.sync.dma_start(mrhs_sb[3:4], lhsT_sb[4:5])
            nc.sync.dma_start(mrhs_sb[4:5], lhsT_sb[3:4])

            # shared coefficient row -> staged[64]
            rc = zp.tile([1, 7, 512], fp32)
            nc.vector.memset(rc[:, 0:3, :], 1.0)
            nc.vector.memset(rc[:, 3:6, :], 2.0)
            nc.vector.memset(rc[:, 6:7, :], 0.0)
            nc.sync.dma_start(staged[64:65], rc[:])

            ones1 = zp.tile([1, 128], fp32)
            ones3 = zp.tile([3, 1], fp32)
            twos3 = zp.tile([3, 1], fp32)
            nc.vector.memset(ones1[:], 1.0)
            nc.vector.memset(ones3[:], 1.0)
            nc.vector.memset(twos3[:], 2.0)

            # one-hot face matrices per hand s, corner k, K-chunk kk
            oh = zp.tile([128, 2, 3, 2, 512], fp32)
            with tc.tile_pool(name="ph0bc", bufs=1, space="PSUM") as bp:
                bc = bp.tile([128, 3, 512], fp32)
                for s in range(2):
                    for k in range(3):
                        nc.tensor.matmul(bc[:, k, :], ones1[:],
                                         faces_sb[:, s, k, :])
                    for k in range(3):
                        for kk in range(2):
                            nc.vector.tensor_scalar(
                                oh[:, s, k, kk, :], bc[:, k, :],
                                cst_sb[:, kk:kk + 1], None, OP.is_equal)

            Ac = zp.tile([3, 512], fp32)
            Bc = zp.tile([3, 512], fp32)
            Cc = zp.tile([3, 512], fp32)
            E1 = zp.tile([3, 512], fp32)
            E2 = zp.tile([3, 512], fp32)
            rotc = zp.tile([3, 4, 512], fp32)
            t1 = zp.tile([3, 512], fp32)
            t2 = zp.tile([3, 512], fp32)
            NN = zp.tile([3, 512], fp32)
            PRD = zp.tile([3, 7, 512], fp32)
            MID = zp.tile([3, 3, 512], fp32)
            C3r = zp.tile([1, 7, 512], fp32)

            with tc.tile_pool(name="ph0ps", bufs=1, space="PSUM") as pp0:
                crn = [pp0.tile([3, 512], fp32, name=f"crn{t}", tag=t)
                       for t in "abc"]
                rot = pp0.tile([3, 4, 512], fp32)
                c3p = pp0.tile([1, 512], fp32)
                for bd in range(NBD):
                    other = bd ^ 1
                    s = other & 1
                    for k in range(3):
                        nc.tensor.matmul(crn[k][:], pts_sb[:, 0, other, :],
                                         oh[:, s, k, 0, :],
                                         start=True, stop=False)
                        nc.tensor.matmul(crn[k][:], pts_sb[:, 1, other, :],
                                         oh[:, s, k, 1, :],
                                         start=False, stop=True)
                    nc.scalar.activation(Ac[:], crn[0][:], AF.Copy)
                    nc.scalar.activation(Bc[:], crn[1][:], AF.Copy)
                    nc.scalar.activation(Cc[:], crn[2][:], AF.Copy)
                    nc.vector.tensor_tensor(E1[:], Bc[:], Ac[:], OP.subtract)
                    nc.vector.tensor_tensor(E2[:], Cc[:], Ac[:], OP.subtract)
                    # n = E1 x E2 via rotations: rot1/rot2 = P1^T/P2^T
                    nc.tensor.matmul(rot[:, 0, :], cst_sb[0:3, 2:5], E1[:])
                    nc.tensor.matmul(rot[:, 1, :], cst_sb[0:3, 5:8], E2[:])
                    nc.tensor.matmul(rot[:, 2, :], cst_sb[0:3, 5:8], E1[:])
                    nc.tensor.matmul(rot[:, 3, :], cst_sb[0:3, 2:5], E2[:])
                    nc.scalar.activation(rotc[:], rot[:], AF.Copy)
                    nc.vector.tensor_tensor(t1[:], rotc[:, 0, :],
                                            rotc[:, 1, :], OP.mult)
                    nc.vector.tensor_tensor(t2[:], rotc[:, 2, :],
                                            rotc[:, 3, :], OP.mult)
                    nc.vector.tensor_tensor(NN[:], t1[:], t2[:], OP.subtract)
                    # products for the c3 reduces
                    nc.vector.tensor_tensor(PRD[:, 0, :], Ac[:], Ac[:], OP.mult)
                    nc.vector.tensor_tensor(PRD[:, 1, :], Bc[:], Bc[:], OP.mult)
                    nc.vector.tensor_tensor(PRD[:, 2, :], Cc[:], Cc[:], OP.mult)
                    nc.vector.tensor_tensor(PRD[:, 3, :], Ac[:], Bc[:], OP.mult)
                    nc.vector.tensor_tensor(PRD[:, 4, :], Bc[:], Cc[:], OP.mult)
                    nc.vector.tensor_tensor(PRD[:, 5, :], Cc[:], Ac[:], OP.mult)
                    nc.vector.tensor_tensor(PRD[:, 6, :], Ac[:], NN[:], OP.mult)
                    nc.vector.tensor_tensor(MID[:, 0, :], Ac[:], Bc[:], OP.add)
                    nc.vector.tensor_tensor(MID[:, 1, :], Bc[:], Cc[:], OP.add)
                    nc.vector.tensor_tensor(MID[:, 2, :], Cc[:], Ac[:], OP.add)
                    for g in range(7):
                        nc.tensor.matmul(c3p[:], ones3[:] if g < 3 else twos3[:],
                                         PRD[:, g, :])
                        nc.scalar.activation(C3r[:, g, :], c3p[:], AF.Copy)
                    # assemble staged rows via SBUF->SBUF DMA
                    r0 = staged[4 * bd:4 * bd + 3]
                    nc.sync.dma_start(r0[:, 0, :], Ac[:])
                    nc.sync.dma_start(r0[:, 1, :], Bc[:])
                    nc.sync.dma_start(r0[:, 2, :], Cc[:])
                    nc.sync.dma_start(r0[:, 3:6, :], MID[:])
                    nc.sync.dma_start(r0[:, 6, :], NN[:])
                    nc.sync.dma_start(staged[4 * bd + 3:4 * bd + 4], C3r[:])

        # ---------------- phase 1: main compute loop --------------------
        with (
            tc.tile_pool(name="store", bufs=1) as spool,
            tc.tile_pool(name="stage", bufs=2) as stpool,
            tc.tile_pool(name="iface", bufs=2) as ipool,
            tc.tile_pool(name="dve", bufs=1) as vpool,
        ):
            denoms = spool.tile([128, SUPER, 512], fp32)
            tts = spool.tile([128, SUPER, 512], fp32)

            def pass_a(ppool, i, j):
                bd, ch = divmod(i, 2)
                if ch == 0:
                    fstage = stpool.tile([5, 7, 512], fp32, tag="fstage")
                    nc.sync.dma_start(fstage[0:4], staged[4 * bd:4 * bd + 4])
                    nc.sync.dma_start(fstage[4:5], staged[64:65])
                    pass_a.stage = fstage
                fstage = pass_a.stage
                lhs = lhsT_sb[:, bd, ch * 128:(ch + 1) * 128]

                wind = ppool.tile([128, 7, 512], fp32, tag="wind")
                md = ppool.tile([128, 256], fp32, tag="md")

                for g in range(7):
                    nc.tensor.matmul(wind[:, g, :F], lhs, fstage[:, g, :F])
                nc.tensor.matmul(md[:, :P], lhs, mrhs_sb[:, bd ^ 1, :P])

                # min-distance: free-dim min, clamp at 0 (matmul roundoff)
                mind = vpool.tile([128, 1], fp32, tag="mind")
                nc.vector.tensor_reduce(mind[:], md[:, :P], AX, OP.min)
                nc.vector.tensor_scalar(minda[:, i:i + 1], mind[:], 0.0, None,
                                        OP.max)

                # norms: clamp squared lengths at 0, sqrt
                rl = ipool.tile([128, 3, 512], fp32, tag="rl")
                for g in range(3):
                    nc.scalar.activation(rl[:, g, :F], wind[:, g, :F], AF.Relu)
                la = ipool.tile([128, 512], fp32, tag="la")
                lb = ipool.tile([128, 512], fp32, tag="lb")
                lc = ipool.tile([128, 512], fp32, tag="lc")
                nc.scalar.activation(la[:, :F], rl[:, 0, :F], AF.Sqrt)
                nc.scalar.activation(lb[:, :F], rl[:, 1, :F], AF.Sqrt)
                nc.scalar.activation(lc[:, :F], rl[:, 2, :F], AF.Sqrt)
                dets = ipool.tile([128, 512], fp32, tag="dets")
                nc.scalar.mul(dets[:, :F], wind[:, 6, :F], 0.5)

                # denominator chain; wind groups 3..5 hold 2ab/2bc/2ca so
                # fold the x0.5 into the scalar_tensor_tensor ops
                u = vpool.tile([128, 512], fp32, tag="u")
                r4 = vpool.tile([128, 512], fp32, tag="r4")
                s5 = vpool.tile([128, 512], fp32, tag="s5")
                v = vpool.tile([128, 512], fp32, tag="v")
                w = vpool.tile([128, 512], fp32, tag="w")
                t6 = vpool.tile([128, 512], fp32, tag="t6")
                nc.vector.scalar_tensor_tensor(r4[:, :F], wind[:, 4, :F], 0.5,
                                               la[:, :F], OP.mult, OP.mult)
                nc.vector.scalar_tensor_tensor(s5[:, :F], wind[:, 5, :F], 0.5,
                                               lb[:, :F], OP.mult, OP.mult)
                nc.vector.tensor_tensor(u[:, :F], la[:, :F], lb[:, :F], OP.mult)
                nc.vector.scalar_tensor_tensor(v[:, :F], wind[:, 3, :F], 0.5,
                                               u[:, :F], OP.mult, OP.add)

                w_ = w[:, :F]
                nc.vector.tensor_tensor(w_, v[:, :F], lc[:, :F], OP.mult)
                nc.vector.tensor_tensor(t6[:, :F], r4[:, :F], s5[:, :F], OP.add)
                den = denoms[:, j, :F]
                nc.vector.tensor_tensor(den, w_, t6[:, :F], OP.add)

                # half-angle atan2 range reduction: tt = det / (rho + |den|)
                xx = ipool.tile([128, 512], fp32, tag="xx")
                yy = ipool.tile([128, 512], fp32, tag="yy")
                ss = vpool.tile([128, 512], fp32, tag="ss", bufs=2)
                rho = ipool.tile([128, 512], fp32, tag="rho")
                axd = ipool.tile([128, 512], fp32, tag="axd")
                dd = vpool.tile([128, 512], fp32, tag="dd")
                rd = vpool.tile([128, 512], fp32, tag="rd")
                nc.scalar.activation(xx[:, :F], den, AF.Square)
                nc.scalar.activation(yy[:, :F], dets[:, :F], AF.Square)
                nc.vector.scalar_tensor_tensor(ss[:, :F], xx[:, :F], 1e-20,
                                               yy[:, :F], OP.add, OP.add)
                nc.scalar.activation(rho[:, :F], ss[:, :F], AF.Sqrt)
                nc.scalar.activation(axd[:, :F], den, AF.Abs)
                nc.vector.tensor_tensor(dd[:, :F], rho[:, :F], axd[:, :F],
                                        OP.add)
                nc.vector.reciprocal_approx_fast(rd[:, :F], dd[:, :F])
                nc.vector.tensor_tensor(tts[:, j, :F], dets[:, :F], rd[:, :F],
                                        OP.mult)

            def pass_b(i, j):
                den = denoms[:, j, :F]
                tt = tts[:, j, :F]
                sgn = ipool.tile([128, 512], fp32, tag="sgn")
                spi = ipool.tile([128, 512], fp32, tag="spi")
                atn = ipool.tile([128, 512], fp32, tag="atn")
                c0 = vpool.tile([128, 512], fp32, tag="c0")
                c1 = vpool.tile([128, 512], fp32, tag="c1")
                sd = vpool.tile([128, 512], fp32, tag="sd")
                nc.scalar.activation(sgn[:, :F], tt, AF.Sign)
                nc.scalar.mul(spi[:, :F], sgn[:, :F], HALF_PI)
                nc.scalar.activation(atn[:, :F], tt, AF.Arctan)
                # half = atn + [den<0]*(pi/2*sign(det) - 2*atn)
                nc.vector.scalar_tensor_tensor(c0[:, :F], atn[:, :F], -2.0,
                                               spi[:, :F], OP.mult, OP.add)
                nc.vector.scalar_tensor_tensor(c1[:, :F], den, 0.0,
                                               c0[:, :F], OP.is_lt, OP.mult)
                nc.vector.scalar_tensor_tensor(sd[:, :F], atn[:, :F], 0.0,
                                               c1[:, :F], OP.add, OP.add,
                                               accum_out=sacc[:, i:i + 1])

            with tc.tile_pool(name="psum", bufs=1, space="PSUM") as ppool:
                for sg in range(NBLK // SUPER):
                    for j in range(SUPER):
                        pass_a(ppool, sg * SUPER + j, j)
                    tc.no_sync_barrier()
                    for j in range(SUPER):
                        pass_b(sg * SUPER + j, j)
                    tc.no_sync_barrier()

            # ------------- final: depth * inside, partition-reduce -------
            inside = cpool.tile([128, NBLK], fp32)
            depth = cpool.tile([128, NBLK], fp32)
            contrib = cpool.tile([128, NBLK], fp32)
            nc.vector.tensor_scalar(inside[:], sacc[:], HALF_PI, None,
                                    OP.is_gt)
            nc.scalar.activation(depth[:], minda[:], AF.Sqrt, bias=beps[:])
            nc.vector.tensor_tensor(contrib[:], depth[:], inside[:], OP.mult)

            with tc.tile_pool(name="psum2", bufs=1, space="PSUM") as p2:
                lpsum = p2.tile([NBLK, 1], fp32)
                nc.tensor.matmul(lpsum[:], contrib[:], ones[:])
                loss_sb = cpool.tile([NBLK, 1], fp32)
                nc.scalar.activation(loss_sb[:], lpsum[:], AF.Copy)
                nc.sync.dma_start(loss_d[:], loss_sb[:])


def _build():
    global _compiled
    if _compiled is not None:
        return _compiled
    import concourse.bacc as bacc
    import concourse.mybir as mybir
    import concourse.tile as tile

    nc = bacc.Bacc("TRN2", target_bir_lowering=False, debug=False,
                   num_devices=NCORES)
    fp32 = mybir.dt.float32
    lhsT_d = nc.dram_tensor("lhsT", (5, NBD, PPAD), fp32, kind="ExternalInput").ap()
    pts_d = nc.dram_tensor("pts", (128, 2, NBD, 3), fp32, kind="ExternalInput").ap()
    faces_d = nc.dram_tensor("faces", (1, 2, 3, 512), fp32, kind="ExternalInput").ap()
    cst_d = nc.dram_tensor("cst", (128, 8), fp32, kind="ExternalInput").ap()
    loss_d = nc.dram_tensor("loss", (NBLK, 1), fp32, kind="ExternalOutput").ap()

    with tile.TileContext(nc) as tc:
        _kernel_body(tc, lhsT_d, pts_d, faces_d, cst_d, loss_d)
    nc.compile()
    _compiled = nc
    return nc


# --------------------------------------------------------------------------
# cached jitted runner + entry point
# --------------------------------------------------------------------------

def _build_runner():
    global _runner
    if _runner is not None:
        return _runner
    import jax
    from jax.sharding import Mesh, PartitionSpec
    from jax.experimental.shard_map import shard_map
    import concourse.mybir as mybir
    from concourse.bass2jax import (_bass_exec_p, partition_id_tensor,
                                    install_neuronx_cc_hook)

    nc = _build()
    install_neuronx_cc_hook()
    pname = nc.partition_id_tensor.name if nc.partition_id_tensor else None
    in_names, out_names, out_avals, zero_outs = [], [], [], []
    for alloc in nc.m.functions[0].allocations:
        if not isinstance(alloc, mybir.MemoryLocationSet):
            continue
        name = alloc.memorylocations[0].name
        if alloc.kind == "ExternalInput":
            if name != pname:
                in_names.append(name)
        elif alloc.kind == "ExternalOutput":
            out_names.append(name)
            shape = tuple(alloc.tensor_shape)
            dtype = mybir.dt.np(alloc.dtype)
            out_avals.append(jax.core.ShapedArray(shape, dtype))
            zero_outs.append(np.zeros(shape, dtype))
    n_params, n_outs = len(in_names), len(out_avals)
    in_names_full = in_names + out_names + ([pname] if pname else [])

    def _body(*args):
        operands = list(args)
        if pname is not None:
            operands.append(partition_id_tensor())
        return tuple(_bass_exec_p.bind(
            *operands, out_avals=tuple(out_avals), in_names=tuple(in_names_full),
            out_names=tuple(out_names), lowering_input_output_aliases=(),
            sim_require_finite=True, sim_require_nnan=True, nc=nc))

    devices = jax.devices()[:NCORES]
    mesh = Mesh(np.asarray(devices), ("core",))
    in_specs = (PartitionSpec("core"),) * (n_params + n_outs)
    out_specs = (PartitionSpec("core"),) * len(out_names)
    sharded = jax.jit(
        shard_map(_body, mesh=mesh, in_specs=in_specs, out_specs=out_specs,
                  check_rep=False),
        donate_argnums=tuple(range(n_params, n_params + n_outs)),
        keep_unused=True)
    czero_shapes = [((NCORES * z.shape[0],) + z.shape[1:], z.dtype)
                    for z in zero_outs]
    _runner = (sharded, in_names, czero_shapes)
    return _runner


def kernel(**inputs) -> np.ndarray:
    global last_exec_time_ns
    lhsT, pts, faces, cst = _host_prep(inputs)
    sharded, in_names, czero_shapes = _build_runner()
    maps = _in_maps(lhsT, pts, faces, cst)
    concat_in = [np.concatenate([maps[c][nm] for c in range(NCORES)], axis=0)
                 for nm in in_names]
    zeros = [np.zeros(s, d) for s, d in czero_shapes]
    out = sharded(*concat_in, *zeros)
    last_exec_time_ns = None

    o0 = np.asarray(out[0]).reshape(NCORES, NBLK)
    loss = np.zeros(B, np.float32)
    for c in range(NCORES):
        # block i = (b_loc*2 + dir)*2 + chunk
        loss[c * NB:(c + 1) * NB] = o0[c].reshape(NB, 4).sum(axis=1)
    return loss


# revision 12
# speedup vs baseline: 6.9848x; 1.0800x over previous
"""Trainium2 Bass kernel for nn_HandIntersectionLoss.

Strategy
--------
Pure data parallel over batch: 64 batches -> 8 cores x 8 local batches.

Wall-clock per call is dominated by the axon tunnel, so the host ships
only the gathered hand points (~140KB/core) and the device derives all
per-(batch,face) matmul constants itself:

  phase 0 (device):
    - one-hot face matrices from f32 face indices (K=1 broadcast matmul
      + is_equal against shipped iota columns)
    - triangle corners A,B,C per (batch,dir) via 2-chunk accumulated
      gather matmuls:  corners[3,500] = pts[128,3]^T @ onehot[128,500]
    - edges E1=B-A, E2=C-A; normal n = E1 x E2 via permutation-matmul
      rotations (engines cannot read partition offsets != 0)
    - dots |A|^2,.., 2A.B,.., 2A.n via ones/twos-column reduce matmuls
    - constants assembled into a persistent `staged` SBUF tile
      ([65,7,512]: 4 rows per (batch,dir) + shared coefficient row)
      via SBUF->SBUF DMAs (the only legal cross-partition mover)

  phase 1 (device): the proven compute loop. Per 128-point block:
    K=5 matmuls against staged constants produce la^2,lb^2,lc^2,
    2ab,2bc,2ca, 2det for [128 points x 500 faces]; per-element chain
    (denominator + range-reduced atan2) on DVE/ACT:

      atan2(det, den) = 2*atan(det / (rho + |den|))            (den >= 0)
                      = sign(det)*pi - 2*atan(det/(rho+|den|)) (den < 0)
      rho = sqrt(det^2 + den^2 + 1e-20)

    inside(p) <=> sum_f half > pi/2.  Min-distance via the same matmul
    trick against derived vert constants (mrhs) + free-dim min-reduce.
    Scalar-engine table sets force the two-pass structure (sqrt vs
    arctan live in different ACT table sets), staged in super-groups.

The jitted shard_map callable is cached across kernel() calls so repeat
calls skip jax retrace/XLA recompile entirely.

Group semantics (raw, no halving on device):
  g0..2: xyz=A|B|C,       c3=|A|^2..,  w=1
  g3..5: xyz=(A+B)..raw,  c3=2A.B..,   w=2   -> col = 2*(A-p).(B-p)
  g6:    xyz=n raw,       c3=2*A.n,    w=0   -> col = 2*det
pass_a compensates with x0.5 folded into existing scalar_tensor_tensor.
"""
import os
import sys
import numpy as np

sys.path.insert(0, '/opt/trn_rl_repo')

B, V_FULL, V_HAND, V_LOOP, N_FACES = 64, 6890, 250, 20, 500
P = V_HAND + 1          # 251 points/verts per hand (incl. lid)
PPAD = 256
NCORES = 8
NB = B // NCORES        # local batches per core
NBD = NB * 2            # (batch, dir) pairs per core
NBLK = NBD * 2          # blocks per core: x2 point-chunks of 128
SUPER = 8               # blocks per two-pass super-group
F = N_FACES
HALF_PI = float(np.pi / 2)

_compiled = None
_runner = None
last_exec_time_ns = None


# --------------------------------------------------------------------------
# host prep: index gathers only (all heavy constant math moved on-device)
# --------------------------------------------------------------------------

def _host_prep(inputs):
    verts = np.asarray(inputs['verts_batch'], dtype=np.float32)
    hi = [np.asarray(inputs['hand_verts_inds_left']),
          np.asarray(inputs['hand_verts_inds_right'])]
    li = [np.asarray(inputs['hand_loop_verts_inds_left']),
          np.asarray(inputs['hand_loop_verts_inds_right'])]
    fc = [np.asarray(inputs['hand_faces_left']),
          np.asarray(inputs['hand_faces_right'])]

    pts = np.full((B, 2, PPAD, 3), 1e2, np.float32)
    for d in range(2):
        pts[:, d, :V_HAND] = verts[:, hi[d]]
        pts[:, d, V_HAND] = verts[:, li[d]].mean(axis=1, dtype=np.float32)

    faces = np.full((1, 2, 3, 512), 300.0, np.float32)
    for s in range(2):
        faces[0, s, :, :F] = fc[s].T.astype(np.float32)

    cst = np.zeros((128, 8), np.float32)
    cst[:, 0] = np.arange(128, dtype=np.float32)
    cst[:, 1] = np.arange(128, 256, dtype=np.float32)
    for m in range(3):
        cst[(m + 1) % 3, 2 + m] = 1.0      # P1 (rot1)
        cst[(m + 2) % 3, 5 + m] = 1.0      # P2 (rot2)
    extra = np.arange(PPAD, dtype=np.float32).reshape(1, PPAD)
    return pts, faces, cst, extra


def _in_maps(pts, faces, cst, extra):
    maps = []
    for c in range(NCORES):
        bs = slice(c * NB, (c + 1) * NB)
        pt = pts[bs].reshape(NBD, 2, 128, 3).transpose(2, 1, 0, 3)
        maps.append({
            "pts": np.ascontiguousarray(pt),
            "faces": faces,
            "cst": cst,
            "extra": extra,
        })
    return maps


# --------------------------------------------------------------------------
# device kernel
# --------------------------------------------------------------------------

def _kernel_body(tc, pts_d, faces_d, cst_d, extra_d, loss_d):
    import concourse.mybir as mybir
    nc = tc.nc
    fp32 = mybir.dt.float32
    AF = mybir.ActivationFunctionType
    OP = mybir.AluOpType
    AX = mybir.AxisListType.X

    with tc.tile_pool(name="const", bufs=1) as cpool:
        lhsT_sb = cpool.tile([5, NBD, PPAD], fp32)
        mrhs_sb = cpool.tile([5, NBD, PPAD], fp32)
        staged = cpool.tile([65, 7, 512], fp32)
        ones = cpool.tile([128, 1], fp32)
        beps = cpool.tile([128, 1], fp32)
        sacc = cpool.tile([128, NBLK], fp32)
        minda = cpool.tile([128, NBLK], fp32)
        nc.vector.memset(ones[:], 1.0)
        nc.vector.memset(beps[:], 1e-12)

        # ---------------- phase 0: derive constants on device ----------
        with tc.tile_pool(name="ph0", bufs=1) as zp:
            ones1 = zp.tile([1, 128], fp32)
            ones3 = zp.tile([3, 1], fp32)
            twos3 = zp.tile([3, 1], fp32)
            nc.vector.memset(ones1[:], 1.0)
            nc.vector.memset(ones3[:], 1.0)
            nc.vector.memset(twos3[:], 2.0)
            pts_sb = zp.tile([128, 2, NBD, 3], fp32)
            faces_sb = zp.tile([1, 2, 3, 512], fp32)
            cst_sb = zp.tile([128, 8], fp32)
            extra_sb = zp.tile([1, PPAD], fp32)
            nc.sync.dma_start(pts_sb[:], pts_d[:])
            nc.sync.dma_start(faces_sb[:], faces_d[:])
            nc.sync.dma_start(cst_sb[:], cst_d[:])
            nc.sync.dma_start(extra_sb[:], extra_d[:])

            # shared coefficient row -> staged[64]
            rc = zp.tile([1, 7, 512], fp32)
            nc.vector.memset(rc[:, 0:3, :], 1.0)
            nc.vector.memset(rc[:, 3:6, :], 2.0)
            nc.vector.memset(rc[:, 6:7, :], 0.0)
            nc.sync.dma_start(staged[64:65], rc[:])

            # one-hot face matrices per hand s, corner k, K-chunk kk
            # + identity one-hot (for pts transposition via gather matmul)
            oh = zp.tile([128, 2, 3, 2, 512], fp32)
            idh = zp.tile([128, 2, PPAD], fp32)
            PT = zp.tile([3, PPAD], fp32)
            SQ = zp.tile([3, PPAD], fp32)
            sqrow = zp.tile([1, NBD, PPAD], fp32)
            onesrow = zp.tile([1, NBD, PPAD], fp32)
            nc.vector.memset(onesrow[:], 1.0)
            with tc.tile_pool(name="ph0bc", bufs=1, space="PSUM") as bp:
                bc = bp.tile([128, 3, 512], fp32)
                bcid = bp.tile([128, PPAD], fp32)
                ptp = bp.tile([3, PPAD], fp32)
                sqp = bp.tile([1, PPAD], fp32)
                for s in range(2):
                    for k in range(3):
                        nc.tensor.matmul(bc[:, k, :], ones1[:],
                                         faces_sb[:, s, k, :])
                    for k in range(3):
                        for kk in range(2):
                            nc.vector.tensor_scalar(
                                oh[:, s, k, kk, :], bc[:, k, :],
                                cst_sb[:, kk:kk + 1], None, OP.is_equal)
                nc.tensor.matmul(bcid[:], ones1[:], extra_sb[:])
                for kk in range(2):
                    nc.vector.tensor_scalar(idh[:, kk, :], bcid[:],
                                            cst_sb[:, kk:kk + 1], None,
                                            OP.is_equal)
                # lhsT rows from pts: -2*pts^T via identity-gather matmuls,
                # |p|^2 via square + ones3-reduce
                for bd in range(NBD):
                    nc.tensor.matmul(ptp[:], pts_sb[:, 0, bd, :],
                                     idh[:, 0, :], start=True, stop=False)
                    nc.tensor.matmul(ptp[:], pts_sb[:, 1, bd, :],
                                     idh[:, 1, :], start=False, stop=True)
                    nc.scalar.mul(lhsT_sb[0:3, bd, :], ptp[:], -2.0)
                    nc.scalar.activation(PT[:], ptp[:], AF.Copy)
                    nc.vector.tensor_tensor(SQ[:], PT[:], PT[:], OP.mult)
                    nc.tensor.matmul(sqp[:], ones3[:], SQ[:])
                    nc.scalar.activation(sqrow[:, bd, :], sqp[:], AF.Copy)
            nc.sync.dma_start(lhsT_sb[3:4], onesrow[:])
            nc.sync.dma_start(lhsT_sb[4:5], sqrow[:])

            # mrhs: rows0..2 = -0.5*lhsT rows0..2 (= vert xyz),
            # row3 <- lhsT row4 (|v|^2), row4 <- lhsT row3 (ones)
            nc.vector.tensor_scalar(mrhs_sb[0:3], lhsT_sb[0:3], -0.5, None,
                                    OP.mult)
            nc.sync.dma_start(mrhs_sb[3:4], lhsT_sb[4:5])
            nc.sync.dma_start(mrhs_sb[4:5], lhsT_sb[3:4])

            Ac = zp.tile([3, 512], fp32)
            Bc = zp.tile([3, 512], fp32)
            Cc = zp.tile([3, 512], fp32)
            E1 = zp.tile([3, 512], fp32)
            E2 = zp.tile([3, 512], fp32)
            rotc = zp.tile([3, 4, 512], fp32)
            t1 = zp.tile([3, 512], fp32)
            t2 = zp.tile([3, 512], fp32)
            NN = zp.tile([3, 512], fp32)
            PRD = zp.tile([3, 7, 512], fp32)
            MID = zp.tile([3, 3, 512], fp32)
            C3r = zp.tile([1, 7, 512], fp32)

            with tc.tile_pool(name="ph0ps", bufs=1, space="PSUM") as pp0:
                crn = [pp0.tile([3, 512], fp32, name=f"crn{t}", tag=t)
                       for t in "abc"]
                rot = pp0.tile([3, 4, 512], fp32)
                c3p = pp0.tile([1, 512], fp32)
                for bd in range(NBD):
                    other = bd ^ 1
                    s = other & 1
                    for k in range(3):
                        nc.tensor.matmul(crn[k][:], pts_sb[:, 0, other, :],
                                         oh[:, s, k, 0, :],
                                         start=True, stop=False)
                        nc.tensor.matmul(crn[k][:], pts_sb[:, 1, other, :],
                                         oh[:, s, k, 1, :],
                                         start=False, stop=True)
                    nc.scalar.activation(Ac[:], crn[0][:], AF.Copy)
                    nc.scalar.activation(Bc[:], crn[1][:], AF.Copy)
                    nc.scalar.activation(Cc[:], crn[2][:], AF.Copy)
                    nc.vector.tensor_tensor(E1[:], Bc[:], Ac[:], OP.subtract)
                    nc.vector.tensor_tensor(E2[:], Cc[:], Ac[:], OP.subtract)
                    # n = E1 x E2 via rotations: rot1/rot2 = P1^T/P2^T
                    nc.tensor.matmul(rot[:, 0, :], cst_sb[0:3, 2:5], E1[:])
                    nc.tensor.matmul(rot[:, 1, :], cst_sb[0:3, 5:8], E2[:])
                    nc.tensor.matmul(rot[:, 2, :], cst_sb[0:3, 5:8], E1[:])
                    nc.tensor.matmul(rot[:, 3, :], cst_sb[0:3, 2:5], E2[:])
                    nc.scalar.activation(rotc[:], rot[:], AF.Copy)
                    nc.vector.tensor_tensor(t1[:], rotc[:, 0, :],
                                            rotc[:, 1, :], OP.mult)
                    nc.vector.tensor_tensor(t2[:], rotc[:, 2, :],
                                            rotc[:, 3, :], OP.mult)
                    nc.vector.tensor_tensor(NN[:], t1[:], t2[:], OP.subtract)
                    # products for the c3 reduces
                    nc.vector.tensor_tensor(PRD[:, 0, :], Ac[:], Ac[:], OP.mult)
                    nc.vector.tensor_tensor(PRD[:, 1, :], Bc[:], Bc[:], OP.mult)
                    nc.vector.tensor_tensor(PRD[:, 2, :], Cc[:], Cc[:], OP.mult)
                    nc.vector.tensor_tensor(PRD[:, 3, :], Ac[:], Bc[:], OP.mult)
                    nc.vector.tensor_tensor(PRD[:, 4, :], Bc[:], Cc[:], OP.mult)
                    nc.vector.tensor_tensor(PRD[:, 5, :], Cc[:], Ac[:], OP.mult)
                    nc.vector.tensor_tensor(PRD[:, 6, :], Ac[:], NN[:], OP.mult)
                    nc.vector.tensor_tensor(MID[:, 0, :], Ac[:], Bc[:], OP.add)
                    nc.vector.tensor_tensor(MID[:, 1, :], Bc[:], Cc[:], OP.add)
                    nc.vector.tensor_tensor(MID[:, 2, :], Cc[:], Ac[:], OP.add)
                    for g in range(7):
                        nc.tensor.matmul(c3p[:], ones3[:] if g < 3 else twos3[:],
                                         PRD[:, g, :])
                        nc.scalar.activation(C3r[:, g, :], c3p[:], AF.Copy)
                    # assemble staged rows via SBUF->SBUF DMA
                    r0 = staged[4 * bd:4 * bd + 3]
                    nc.sync.dma_start(r0[:, 0, :], Ac[:])
                    nc.sync.dma_start(r0[:, 1, :], Bc[:])
                    nc.sync.dma_start(r0[:, 2, :], Cc[:])
                    nc.sync.dma_start(r0[:, 3:6, :], MID[:])
                    nc.sync.dma_start(r0[:, 6, :], NN[:])
                    nc.sync.dma_start(staged[4 * bd + 3:4 * bd + 4], C3r[:])

        # ---------------- phase 1: main compute loop --------------------
        with (
            tc.tile_pool(name="store", bufs=1) as spool,
            tc.tile_pool(name="stage", bufs=2) as stpool,
            tc.tile_pool(name="iface", bufs=2) as ipool,
            tc.tile_pool(name="dve", bufs=1) as vpool,
        ):
            denoms = spool.tile([128, SUPER, 512], fp32)
            tts = spool.tile([128, SUPER, 512], fp32)

            def pass_a(ppool, i, j):
                bd, ch = divmod(i, 2)
                if ch == 0:
                    fstage = stpool.tile([5, 7, 512], fp32, tag="fstage")
                    nc.sync.dma_start(fstage[0:4], staged[4 * bd:4 * bd + 4])
                    nc.sync.dma_start(fstage[4:5], staged[64:65])
                    pass_a.stage = fstage
                fstage = pass_a.stage
                lhs = lhsT_sb[:, bd, ch * 128:(ch + 1) * 128]

                wind = ppool.tile([128, 7, 512], fp32, tag="wind")
                md = ppool.tile([128, 256], fp32, tag="md")

                for g in range(7):
                    nc.tensor.matmul(wind[:, g, :F], lhs, fstage[:, g, :F])
                nc.tensor.matmul(md[:, :P], lhs, mrhs_sb[:, bd ^ 1, :P])

                # min-distance: free-dim min, clamp at 0 (matmul roundoff)
                mind = vpool.tile([128, 1], fp32, tag="mind")
                nc.vector.tensor_reduce(mind[:], md[:, :P], AX, OP.min)
                nc.vector.tensor_scalar(minda[:, i:i + 1], mind[:], 0.0, None,
                                        OP.max)

                # norms: clamp squared lengths at 0, sqrt
                rl = ipool.tile([128, 3, 512], fp32, tag="rl")
                for g in range(3):
                    nc.scalar.activation(rl[:, g, :F], wind[:, g, :F], AF.Relu)
                la = ipool.tile([128, 512], fp32, tag="la")
                lb = ipool.tile([128, 512], fp32, tag="lb")
                lc = ipool.tile([128, 512], fp32, tag="lc")
                nc.scalar.activation(la[:, :F], rl[:, 0, :F], AF.Sqrt)
                nc.scalar.activation(lb[:, :F], rl[:, 1, :F], AF.Sqrt)
                nc.scalar.activation(lc[:, :F], rl[:, 2, :F], AF.Sqrt)
                dets = ipool.tile([128, 512], fp32, tag="dets")
                nc.scalar.mul(dets[:, :F], wind[:, 6, :F], 0.5)

                # denominator chain; wind groups 3..5 hold 2ab/2bc/2ca so
                # fold the x0.5 into the scalar_tensor_tensor ops
                u = vpool.tile([128, 512], fp32, tag="u")
                r4 = vpool.tile([128, 512], fp32, tag="r4")
                s5 = vpool.tile([128, 512], fp32, tag="s5")
                v = vpool.tile([128, 512], fp32, tag="v")
                w = vpool.tile([128, 512], fp32, tag="w")
                t6 = vpool.tile([128, 512], fp32, tag="t6")
                nc.vector.scalar_tensor_tensor(r4[:, :F], wind[:, 4, :F], 0.5,
                                               la[:, :F], OP.mult, OP.mult)
                nc.vector.scalar_tensor_tensor(s5[:, :F], wind[:, 5, :F], 0.5,
                                               lb[:, :F], OP.mult, OP.mult)
                nc.vector.tensor_tensor(u[:, :F], la[:, :F], lb[:, :F], OP.mult)
                nc.vector.scalar_tensor_tensor(v[:, :F], wind[:, 3, :F], 0.5,
                                               u[:, :F], OP.mult, OP.add)

                w_ = w[:, :F]
                nc.vector.tensor_tensor(w_, v[:, :F], lc[:, :F], OP.mult)
                nc.vector.tensor_tensor(t6[:, :F], r4[:, :F], s5[:, :F], OP.add)
                den = denoms[:, j, :F]
                nc.vector.tensor_tensor(den, w_, t6[:, :F], OP.add)

                # half-angle atan2 range reduction: tt = det / (rho + |den|)
                xx = ipool.tile([128, 512], fp32, tag="xx")
                yy = ipool.tile([128, 512], fp32, tag="yy")
                ss = vpool.tile([128, 512], fp32, tag="ss", bufs=2)
                rho = ipool.tile([128, 512], fp32, tag="rho")
                axd = ipool.tile([128, 512], fp32, tag="axd")
                dd = vpool.tile([128, 512], fp32, tag="dd")
                rd = vpool.tile([128, 512], fp32, tag="rd")
                nc.scalar.activation(xx[:, :F], den, AF.Square)
                nc.scalar.activation(yy[:, :F], dets[:, :F], AF.Square)
                nc.vector.scalar_tensor_tensor(ss[:, :F], xx[:, :F], 1e-20,
                                               yy[:, :F], OP.add, OP.add)
                nc.scalar.activation(rho[:, :F], ss[:, :F], AF.Sqrt)
                nc.scalar.activation(axd[:, :F], den, AF.Abs)
                nc.vector.tensor_tensor(dd[:, :F], rho[:, :F], axd[:, :F],
                                        OP.add)
                nc.vector.reciprocal_approx_fast(rd[:, :F], dd[:, :F])
                nc.vector.tensor_tensor(tts[:, j, :F], dets[:, :F], rd[:, :F],
                                        OP.mult)

            def pass_b(i, j):
                den = denoms[:, j, :F]
                tt = tts[:, j, :F]
                sgn = ipool.tile([128, 512], fp32, tag="sgn")
                spi = ipool.tile([128, 512], fp32, tag="spi")
                atn = ipool.tile([128, 512], fp32, tag="atn")
                c0 = vpool.tile([128, 512], fp32, tag="c0")
                c1 = vpool.tile([128, 512], fp32, tag="c1")
                sd = vpool.tile([128, 512], fp32, tag="sd")
                nc.scalar.activation(sgn[:, :F], tt, AF.Sign)
                nc.scalar.mul(spi[:, :F], sgn[:, :F], HALF_PI)
                nc.scalar.activation(atn[:, :F], tt, AF.Arctan)
                # half = atn + [den<0]*(pi/2*sign(det) - 2*atn)
                nc.vector.scalar_tensor_tensor(c0[:, :F], atn[:, :F], -2.0,
                                               spi[:, :F], OP.mult, OP.add)
                nc.vector.scalar_tensor_tensor(c1[:, :F], den, 0.0,
                                               c0[:, :F], OP.is_lt, OP.mult)
                nc.vector.scalar_tensor_tensor(sd[:, :F], atn[:, :F], 0.0,
                                               c1[:, :F], OP.add, OP.add,
                                               accum_out=sacc[:, i:i + 1])

            with tc.tile_pool(name="psum", bufs=1, space="PSUM") as ppool:
                for sg in range(NBLK // SUPER):
                    for j in range(SUPER):
                        pass_a(ppool, sg * SUPER + j, j)
                    tc.no_sync_barrier()
                    for j in range(SUPER):
                        pass_b(sg * SUPER + j, j)
                    tc.no_sync_barrier()

            # ------------- final: depth * inside, partition-reduce -------
            inside = cpool.tile([128, NBLK], fp32)
            depth = cpool.tile([128, NBLK], fp32)
            contrib = cpool.tile([128, NBLK], fp32)
            nc.vector.tensor_scalar(inside[:], sacc[:], HALF_PI, None,
                                    OP.is_gt)
            nc.scalar.activation(depth[:], minda[:], AF.Sqrt, bias=beps[:])
            nc.vector.tensor_tensor(contrib[:], depth[:], inside[:], OP.mult)

            with tc.tile_pool(name="psum2", bufs=1, space="PSUM") as p2:
                lpsum = p2.tile([NBLK, 1], fp32)
                nc.tensor.matmul(lpsum[:], contrib[:], ones[:])
                loss_sb = cpool.tile([NBLK, 1], fp32)
                nc.scalar.activation(loss_sb[:], lpsum[:], AF.Copy)
                nc.sync.dma_start(loss_d[:], loss_sb[:])


def _build():
    global _compiled
    if _compiled is not None:
        return _compiled
    import concourse.bacc as bacc
    import concourse.mybir as mybir
    import concourse.tile as tile

    nc = bacc.Bacc("TRN2", target_bir_lowering=False, debug=False,
                   num_devices=NCORES)
    fp32 = mybir.dt.float32
    pts_d = nc.dram_tensor("pts", (128, 2, NBD, 3), fp32, kind="ExternalInput").ap()
    faces_d = nc.dram_tensor("faces", (1, 2, 3, 512), fp32, kind="ExternalInput").ap()
    cst_d = nc.dram_tensor("cst", (128, 8), fp32, kind="ExternalInput").ap()
    extra_d = nc.dram_tensor("extra", (1, PPAD), fp32, kind="ExternalInput").ap()
    loss_d = nc.dram_tensor("loss", (NBLK, 1), fp32, kind="ExternalOutput").ap()

    with tile.TileContext(nc) as tc:
        _kernel_body(tc, pts_d, faces_d, cst_d, extra_d, loss_d)
    nc.compile()
    _compiled = nc
    return nc


# --------------------------------------------------------------------------
# cached jitted runner + entry point
# --------------------------------------------------------------------------

def _build_runner():
    global _runner
    if _runner is not None:
        return _runner
    import jax
    from jax.sharding import Mesh, PartitionSpec
    from jax.experimental.shard_map import shard_map
    import concourse.mybir as mybir
    from concourse.bass2jax import (_bass_exec_p, partition_id_tensor,
                                    install_neuronx_cc_hook)

    nc = _build()
    install_neuronx_cc_hook()
    pname = nc.partition_id_tensor.name if nc.partition_id_tensor else None
    in_names, out_names, out_avals, zero_outs = [], [], [], []
    for alloc in nc.m.functions[0].allocations:
        if not isinstance(alloc, mybir.MemoryLocationSet):
            continue
        name = alloc.memorylocations[0].name
        if alloc.kind == "ExternalInput":
            if name != pname:
                in_names.append(name)
        elif alloc.kind == "ExternalOutput":
            out_names.append(name)
            shape = tuple(alloc.tensor_shape)
            dtype = mybir.dt.np(alloc.dtype)
            out_avals.append(jax.core.ShapedArray(shape, dtype))
            zero_outs.append(np.zeros(shape, dtype))
    n_params, n_outs = len(in_names), len(out_avals)
    in_names_full = in_names + out_names + ([pname] if pname else [])

    def _body(*args):
        operands = list(args)
        if pname is not None:
            operands.append(partition_id_tensor())
        return tuple(_bass_exec_p.bind(
            *operands, out_avals=tuple(out_avals), in_names=tuple(in_names_full),
            out_names=tuple(out_names), lowering_input_output_aliases=(),
            sim_require_finite=True, sim_require_nnan=True, nc=nc))

    devices = jax.devices()[:NCORES]
    mesh = Mesh(np.asarray(devices), ("core",))
    in_specs = (PartitionSpec("core"),) * (n_params + n_outs)
    out_specs = (PartitionSpec("core"),) * len(out_names)
    sharded = jax.jit(
        shard_map(_body, mesh=mesh, in_specs=in_specs, out_specs=out_specs,
                  check_rep=False),
        donate_argnums=tuple(range(n_params, n_params + n_outs)),
        keep_unused=True)
    czero_shapes = [((NCORES * z.shape[0],) + z.shape[1:], z.dtype)
                    for z in zero_outs]
    _runner = (sharded, in_names, czero_shapes)
    return _runner


def kernel(**inputs) -> np.ndarray:
    global last_exec_time_ns
    pts, faces, cst, extra = _host_prep(inputs)
    sharded, in_names, czero_shapes = _build_runner()
    maps = _in_maps(pts, faces, cst, extra)
    concat_in = [np.concatenate([maps[c][nm] for c in range(NCORES)], axis=0)
                 for nm in in_names]
    zeros = [np.zeros(s, d) for s, d in czero_shapes]
    out = sharded(*concat_in, *zeros)
    last_exec_time_ns = None

    o0 = np.asarray(out[0]).reshape(NCORES, NBLK)
    loss = np.zeros(B, np.float32)
    for c in range(NCORES):
        # block i = (b_loc*2 + dir)*2 + chunk
        loss[c * NB:(c + 1) * NB] = o0[c].reshape(NB, 4).sum(axis=1)
    return loss


# revision 13
# speedup vs baseline: 9.6297x; 1.3787x over previous
"""Trainium2 Bass kernel for nn_HandIntersectionLoss.

Strategy
--------
Pure data parallel over batch: 64 batches -> 8 cores x 8 local batches.

Wall-clock per call is dominated by the axon tunnel, so the host ships
only the gathered hand points (~140KB/core) and the device derives all
per-(batch,face) matmul constants itself:

  phase 0 (device):
    - one-hot face matrices from f32 face indices (K=1 broadcast matmul
      + is_equal against shipped iota columns)
    - triangle corners A,B,C per (batch,dir) via 2-chunk accumulated
      gather matmuls:  corners[3,500] = pts[128,3]^T @ onehot[128,500]
    - edges E1=B-A, E2=C-A; normal n = E1 x E2 via permutation-matmul
      rotations (engines cannot read partition offsets != 0)
    - dots |A|^2,.., 2A.B,.., 2A.n via ones/twos-column reduce matmuls
    - constants assembled into a persistent `staged` SBUF tile
      ([65,7,512]: 4 rows per (batch,dir) + shared coefficient row)
      via SBUF->SBUF DMAs (the only legal cross-partition mover)

  phase 1 (device): the proven compute loop. Per 128-point block:
    K=5 matmuls against staged constants produce la^2,lb^2,lc^2,
    2ab,2bc,2ca, 2det for [128 points x 500 faces]; per-element chain
    (denominator + range-reduced atan2) on DVE/ACT:

      atan2(det, den) = 2*atan(det / (rho + |den|))            (den >= 0)
                      = sign(det)*pi - 2*atan(det/(rho+|den|)) (den < 0)
      rho = sqrt(det^2 + den^2 + 1e-20)

    inside(p) <=> sum_f half > pi/2.  Min-distance via the same matmul
    trick against derived vert constants (mrhs) + free-dim min-reduce.
    Scalar-engine table sets force the two-pass structure (sqrt vs
    arctan live in different ACT table sets), staged in super-groups.

The jitted shard_map callable is cached across kernel() calls so repeat
calls skip jax retrace/XLA recompile entirely.

Group semantics (raw, no halving on device):
  g0..2: xyz=A|B|C,       c3=|A|^2..,  w=1
  g3..5: xyz=(A+B)..raw,  c3=2A.B..,   w=2   -> col = 2*(A-p).(B-p)
  g6:    xyz=n raw,       c3=2*A.n,    w=0   -> col = 2*det
pass_a compensates with x0.5 folded into existing scalar_tensor_tensor.
"""
import os
import sys
import numpy as np

sys.path.insert(0, '/opt/trn_rl_repo')

B, V_FULL, V_HAND, V_LOOP, N_FACES = 64, 6890, 250, 20, 500
P = V_HAND + 1          # 251 points/verts per hand (incl. lid)
PPAD = 256
NCORES = 8
NB = B // NCORES        # local batches per core
NBD = NB * 2            # (batch, dir) pairs per core
NBLK = NBD * 2          # blocks per core: x2 point-chunks of 128
SUPER = 8               # blocks per two-pass super-group
F = N_FACES
HALF_PI = float(np.pi / 2)

_compiled = None
SKIP_P1 = False
_runner = None
last_exec_time_ns = None


# --------------------------------------------------------------------------
# host prep: index gathers only (all heavy constant math moved on-device)
# --------------------------------------------------------------------------

def _host_prep(inputs):
    verts = np.asarray(inputs['verts_batch'], dtype=np.float32)
    hi = [np.asarray(inputs['hand_verts_inds_left']),
          np.asarray(inputs['hand_verts_inds_right'])]
    li = [np.asarray(inputs['hand_loop_verts_inds_left']),
          np.asarray(inputs['hand_loop_verts_inds_right'])]
    fc = [np.asarray(inputs['hand_faces_left']),
          np.asarray(inputs['hand_faces_right'])]

    pts = np.full((B, 2, PPAD, 3), 1e2, np.float32)
    for d in range(2):
        pts[:, d, :V_HAND] = verts[:, hi[d]]
        pts[:, d, V_HAND] = verts[:, li[d]].mean(axis=1, dtype=np.float32)

    faces = np.full((1, 2, 3, 512), 300.0, np.float32)
    for s in range(2):
        faces[0, s, :, :F] = fc[s].T.astype(np.float32)

    cst = np.zeros((128, 8), np.float32)
    cst[:, 0] = np.arange(128, dtype=np.float32)
    cst[:, 1] = np.arange(128, 256, dtype=np.float32)
    for m in range(3):
        cst[(m + 1) % 3, 2 + m] = 1.0      # P1 (rot1)
        cst[(m + 2) % 3, 5 + m] = 1.0      # P2 (rot2)
    extra = np.arange(PPAD, dtype=np.float32).reshape(1, PPAD)
    return pts, faces, cst, extra


def _in_maps(pts, faces, cst, extra):
    maps = []
    for c in range(NCORES):
        bs = slice(c * NB, (c + 1) * NB)
        pt = pts[bs].reshape(NBD, 2, 128, 3).transpose(2, 1, 0, 3)
        maps.append({
            "pts": np.ascontiguousarray(pt),
            "faces": faces,
            "cst": cst,
            "extra": extra,
        })
    return maps


# --------------------------------------------------------------------------
# device kernel
# --------------------------------------------------------------------------

def _kernel_body(tc, pts_d, faces_d, cst_d, extra_d, loss_d):
    import concourse.mybir as mybir
    nc = tc.nc
    fp32 = mybir.dt.float32
    AF = mybir.ActivationFunctionType
    OP = mybir.AluOpType
    AX = mybir.AxisListType.X

    with tc.tile_pool(name="const", bufs=1) as cpool:
        lhsT_sb = cpool.tile([5, NBD, PPAD], fp32)
        mrhs_sb = cpool.tile([5, NBD, PPAD], fp32)
        staged = cpool.tile([65, 7, 512], fp32)
        ones = cpool.tile([128, 1], fp32)
        beps = cpool.tile([128, 1], fp32)
        sacc = cpool.tile([128, NBLK], fp32)
        minda = cpool.tile([128, NBLK], fp32)
        nc.vector.memset(ones[:], 1.0)
        nc.vector.memset(beps[:], 1e-12)

        # ---------------- phase 0: derive constants on device ----------
        with tc.tile_pool(name="ph0", bufs=1) as zp:
            ones1 = zp.tile([1, 128], fp32)
            ones3 = zp.tile([3, 1], fp32)
            twos3 = zp.tile([3, 1], fp32)
            nc.vector.memset(ones1[:], 1.0)
            nc.vector.memset(ones3[:], 1.0)
            nc.vector.memset(twos3[:], 2.0)
            pts_sb = zp.tile([128, 2, NBD, 3], fp32)
            faces_sb = zp.tile([1, 2, 3, 512], fp32)
            cst_sb = zp.tile([128, 8], fp32)
            extra_sb = zp.tile([1, PPAD], fp32)
            nc.sync.dma_start(pts_sb[:], pts_d[:])
            nc.sync.dma_start(faces_sb[:], faces_d[:])
            nc.sync.dma_start(cst_sb[:], cst_d[:])
            nc.sync.dma_start(extra_sb[:], extra_d[:])

            # shared coefficient row -> staged[64]
            rc = zp.tile([1, 7, 512], fp32)
            nc.vector.memset(rc[:, 0:3, :], 1.0)
            nc.vector.memset(rc[:, 3:6, :], 2.0)
            nc.vector.memset(rc[:, 6:7, :], 0.0)
            nc.sync.dma_start(staged[64:65], rc[:])

            # one-hot face matrices per hand s, corner k, K-chunk kk
            # + identity one-hot (for pts transposition via gather matmul)
            oh = zp.tile([128, 2, 3, 2, 512], fp32)
            idh = zp.tile([128, 2, PPAD], fp32)
            PT = zp.tile([3, PPAD], fp32)
            SQ = zp.tile([3, PPAD], fp32)
            sqrow = zp.tile([1, NBD, PPAD], fp32)
            onesrow = zp.tile([1, NBD, PPAD], fp32)
            nc.vector.memset(onesrow[:], 1.0)
            with tc.tile_pool(name="ph0bc", bufs=1, space="PSUM") as bp:
                bc = bp.tile([128, 3, 512], fp32)
                bcid = bp.tile([128, PPAD], fp32)
                ptp = bp.tile([3, PPAD], fp32)
                sqp = bp.tile([1, PPAD], fp32)
                for s in range(2):
                    for k in range(3):
                        nc.tensor.matmul(bc[:, k, :], ones1[:],
                                         faces_sb[:, s, k, :])
                    for k in range(3):
                        for kk in range(2):
                            nc.vector.tensor_scalar(
                                oh[:, s, k, kk, :], bc[:, k, :],
                                cst_sb[:, kk:kk + 1], None, OP.is_equal)
                nc.tensor.matmul(bcid[:], ones1[:], extra_sb[:])
                for kk in range(2):
                    nc.vector.tensor_scalar(idh[:, kk, :], bcid[:],
                                            cst_sb[:, kk:kk + 1], None,
                                            OP.is_equal)
                # lhsT rows from pts: -2*pts^T via identity-gather matmuls,
                # |p|^2 via square + ones3-reduce
                for bd in range(NBD):
                    nc.tensor.matmul(ptp[:], pts_sb[:, 0, bd, :],
                                     idh[:, 0, :], start=True, stop=False)
                    nc.tensor.matmul(ptp[:], pts_sb[:, 1, bd, :],
                                     idh[:, 1, :], start=False, stop=True)
                    nc.scalar.mul(lhsT_sb[0:3, bd, :], ptp[:], -2.0)
                    nc.scalar.activation(PT[:], ptp[:], AF.Copy)
                    nc.vector.tensor_tensor(SQ[:], PT[:], PT[:], OP.mult)
                    nc.tensor.matmul(sqp[:], ones3[:], SQ[:])
                    nc.scalar.activation(sqrow[:, bd, :], sqp[:], AF.Copy)
            nc.sync.dma_start(lhsT_sb[3:4], onesrow[:])
            nc.sync.dma_start(lhsT_sb[4:5], sqrow[:])

            # mrhs: rows0..2 = -0.5*lhsT rows0..2 (= vert xyz),
            # row3 <- lhsT row4 (|v|^2), row4 <- lhsT row3 (ones)
            nc.vector.tensor_scalar(mrhs_sb[0:3], lhsT_sb[0:3], -0.5, None,
                                    OP.mult)
            nc.sync.dma_start(mrhs_sb[3:4], lhsT_sb[4:5])
            nc.sync.dma_start(mrhs_sb[4:5], lhsT_sb[3:4])

            Ac = zp.tile([3, 512], fp32)
            Bc = zp.tile([3, 512], fp32)
            Cc = zp.tile([3, 512], fp32)
            E1 = zp.tile([3, 512], fp32)
            E2 = zp.tile([3, 512], fp32)
            rotc = zp.tile([3, 4, 512], fp32)
            t1 = zp.tile([3, 512], fp32)
            t2 = zp.tile([3, 512], fp32)
            NN = zp.tile([3, 512], fp32)
            PRD = zp.tile([3, 7, 512], fp32)
            MID = zp.tile([3, 3, 512], fp32)
            C3r = zp.tile([1, 7, 512], fp32)

            with tc.tile_pool(name="ph0ps", bufs=1, space="PSUM") as pp0:
                crn = [pp0.tile([3, 512], fp32, name=f"crn{t}", tag=t)
                       for t in "abc"]
                rot = pp0.tile([3, 4, 512], fp32)
                c3p = pp0.tile([1, 512], fp32)
                for bd in range(NBD):
                    other = bd ^ 1
                    s = other & 1
                    for k in range(3):
                        nc.tensor.matmul(crn[k][:], pts_sb[:, 0, other, :],
                                         oh[:, s, k, 0, :],
                                         start=True, stop=False)
                        nc.tensor.matmul(crn[k][:], pts_sb[:, 1, other, :],
                                         oh[:, s, k, 1, :],
                                         start=False, stop=True)
                    nc.scalar.activation(Ac[:], crn[0][:], AF.Copy)
                    nc.scalar.activation(Bc[:], crn[1][:], AF.Copy)
                    nc.scalar.activation(Cc[:], crn[2][:], AF.Copy)
                    nc.vector.tensor_tensor(E1[:], Bc[:], Ac[:], OP.subtract)
                    nc.vector.tensor_tensor(E2[:], Cc[:], Ac[:], OP.subtract)
                    # n = E1 x E2 via rotations: rot1/rot2 = P1^T/P2^T
                    nc.tensor.matmul(rot[:, 0, :], cst_sb[0:3, 2:5], E1[:])
                    nc.tensor.matmul(rot[:, 1, :], cst_sb[0:3, 5:8], E2[:])
                    nc.tensor.matmul(rot[:, 2, :], cst_sb[0:3, 5:8], E1[:])
                    nc.tensor.matmul(rot[:, 3, :], cst_sb[0:3, 2:5], E2[:])
                    nc.scalar.activation(rotc[:], rot[:], AF.Copy)
                    nc.vector.tensor_tensor(t1[:], rotc[:, 0, :],
                                            rotc[:, 1, :], OP.mult)
                    nc.vector.tensor_tensor(t2[:], rotc[:, 2, :],
                                            rotc[:, 3, :], OP.mult)
                    nc.vector.tensor_tensor(NN[:], t1[:], t2[:], OP.subtract)
                    # products for the c3 reduces
                    nc.vector.tensor_tensor(PRD[:, 0, :], Ac[:], Ac[:], OP.mult)
                    nc.vector.tensor_tensor(PRD[:, 1, :], Bc[:], Bc[:], OP.mult)
                    nc.vector.tensor_tensor(PRD[:, 2, :], Cc[:], Cc[:], OP.mult)
                    nc.vector.tensor_tensor(PRD[:, 3, :], Ac[:], Bc[:], OP.mult)
                    nc.vector.tensor_tensor(PRD[:, 4, :], Bc[:], Cc[:], OP.mult)
                    nc.vector.tensor_tensor(PRD[:, 5, :], Cc[:], Ac[:], OP.mult)
                    nc.vector.tensor_tensor(PRD[:, 6, :], Ac[:], NN[:], OP.mult)
                    nc.vector.tensor_tensor(MID[:, 0, :], Ac[:], Bc[:], OP.add)
                    nc.vector.tensor_tensor(MID[:, 1, :], Bc[:], Cc[:], OP.add)
                    nc.vector.tensor_tensor(MID[:, 2, :], Cc[:], Ac[:], OP.add)
                    for g in range(7):
                        nc.tensor.matmul(c3p[:], ones3[:] if g < 3 else twos3[:],
                                         PRD[:, g, :])
                        nc.scalar.activation(C3r[:, g, :], c3p[:], AF.Copy)
                    # assemble staged rows via SBUF->SBUF DMA
                    r0 = staged[4 * bd:4 * bd + 3]
                    nc.sync.dma_start(r0[:, 0, :], Ac[:])
                    nc.sync.dma_start(r0[:, 1, :], Bc[:])
                    nc.sync.dma_start(r0[:, 2, :], Cc[:])
                    nc.sync.dma_start(r0[:, 3:6, :], MID[:])
                    nc.sync.dma_start(r0[:, 6, :], NN[:])
                    nc.sync.dma_start(staged[4 * bd + 3:4 * bd + 4], C3r[:])

        # ---------------- phase 1: main compute loop --------------------
        with (
            tc.tile_pool(name="store", bufs=1) as spool,
            tc.tile_pool(name="stage", bufs=2) as stpool,
            tc.tile_pool(name="iface", bufs=2) as ipool,
            tc.tile_pool(name="dve", bufs=1) as vpool,
        ):
            denoms = spool.tile([128, SUPER, 512], fp32)
            tts = spool.tile([128, SUPER, 512], fp32)

            def pass_a(ppool, i, j):
                bd, ch = divmod(i, 2)
                if ch == 0:
                    fstage = stpool.tile([5, 7, 512], fp32, tag="fstage")
                    nc.sync.dma_start(fstage[0:4], staged[4 * bd:4 * bd + 4])
                    nc.sync.dma_start(fstage[4:5], staged[64:65])
                    pass_a.stage = fstage
                fstage = pass_a.stage
                lhs = lhsT_sb[:, bd, ch * 128:(ch + 1) * 128]

                wind = ppool.tile([128, 7, 512], fp32, tag="wind")
                md = ppool.tile([128, 256], fp32, tag="md")

                for g in range(7):
                    nc.tensor.matmul(wind[:, g, :F], lhs, fstage[:, g, :F])
                nc.tensor.matmul(md[:, :P], lhs, mrhs_sb[:, bd ^ 1, :P])

                # min-distance: free-dim min, clamp at 0 (matmul roundoff)
                mind = vpool.tile([128, 1], fp32, tag="mind")
                nc.vector.tensor_reduce(mind[:], md[:, :P], AX, OP.min)
                nc.vector.tensor_scalar(minda[:, i:i + 1], mind[:], 0.0, None,
                                        OP.max)

                # norms: clamp squared lengths at 0, sqrt
                rl = ipool.tile([128, 3, 512], fp32, tag="rl")
                for g in range(3):
                    nc.scalar.activation(rl[:, g, :F], wind[:, g, :F], AF.Relu)
                la = ipool.tile([128, 512], fp32, tag="la")
                lb = ipool.tile([128, 512], fp32, tag="lb")
                lc = ipool.tile([128, 512], fp32, tag="lc")
                nc.scalar.activation(la[:, :F], rl[:, 0, :F], AF.Sqrt)
                nc.scalar.activation(lb[:, :F], rl[:, 1, :F], AF.Sqrt)
                nc.scalar.activation(lc[:, :F], rl[:, 2, :F], AF.Sqrt)
                dets = ipool.tile([128, 512], fp32, tag="dets")
                nc.scalar.mul(dets[:, :F], wind[:, 6, :F], 0.5)

                # denominator chain; wind groups 3..5 hold 2ab/2bc/2ca so
                # fold the x0.5 into the scalar_tensor_tensor ops
                u = vpool.tile([128, 512], fp32, tag="u")
                r4 = vpool.tile([128, 512], fp32, tag="r4")
                s5 = vpool.tile([128, 512], fp32, tag="s5")
                v = vpool.tile([128, 512], fp32, tag="v")
                w = vpool.tile([128, 512], fp32, tag="w")
                t6 = vpool.tile([128, 512], fp32, tag="t6")
                nc.vector.scalar_tensor_tensor(r4[:, :F], wind[:, 4, :F], 0.5,
                                               la[:, :F], OP.mult, OP.mult)
                nc.vector.scalar_tensor_tensor(s5[:, :F], wind[:, 5, :F], 0.5,
                                               lb[:, :F], OP.mult, OP.mult)
                nc.vector.tensor_tensor(u[:, :F], la[:, :F], lb[:, :F], OP.mult)
                nc.vector.scalar_tensor_tensor(v[:, :F], wind[:, 3, :F], 0.5,
                                               u[:, :F], OP.mult, OP.add)

                w_ = w[:, :F]
                nc.vector.tensor_tensor(w_, v[:, :F], lc[:, :F], OP.mult)
                nc.vector.tensor_tensor(t6[:, :F], r4[:, :F], s5[:, :F], OP.add)
                den = denoms[:, j, :F]
                nc.vector.tensor_tensor(den, w_, t6[:, :F], OP.add)

                # half-angle atan2 range reduction: tt = det / (rho + |den|)
                xx = ipool.tile([128, 512], fp32, tag="xx")
                yy = ipool.tile([128, 512], fp32, tag="yy")
                ss = vpool.tile([128, 512], fp32, tag="ss", bufs=2)
                rho = ipool.tile([128, 512], fp32, tag="rho")
                axd = ipool.tile([128, 512], fp32, tag="axd")
                dd = vpool.tile([128, 512], fp32, tag="dd")
                rd = vpool.tile([128, 512], fp32, tag="rd")
                nc.scalar.activation(xx[:, :F], den, AF.Square)
                nc.scalar.activation(yy[:, :F], dets[:, :F], AF.Square)
                nc.vector.scalar_tensor_tensor(ss[:, :F], xx[:, :F], 1e-20,
                                               yy[:, :F], OP.add, OP.add)
                nc.scalar.activation(rho[:, :F], ss[:, :F], AF.Sqrt)
                nc.scalar.activation(axd[:, :F], den, AF.Abs)
                nc.vector.tensor_tensor(dd[:, :F], rho[:, :F], axd[:, :F],
                                        OP.add)
                nc.vector.reciprocal_approx_fast(rd[:, :F], dd[:, :F])
                nc.vector.tensor_tensor(tts[:, j, :F], dets[:, :F], rd[:, :F],
                                        OP.mult)

            def pass_b(i, j):
                den = denoms[:, j, :F]
                tt = tts[:, j, :F]
                sgn = ipool.tile([128, 512], fp32, tag="sgn")
                spi = ipool.tile([128, 512], fp32, tag="spi")
                atn = ipool.tile([128, 512], fp32, tag="atn")
                c0 = vpool.tile([128, 512], fp32, tag="c0")
                c1 = vpool.tile([128, 512], fp32, tag="c1")
                sd = vpool.tile([128, 512], fp32, tag="sd")
                nc.scalar.activation(sgn[:, :F], tt, AF.Sign)
                nc.scalar.mul(spi[:, :F], sgn[:, :F], HALF_PI)
                nc.scalar.activation(atn[:, :F], tt, AF.Arctan)
                # half = atn + [den<0]*(pi/2*sign(det) - 2*atn)
                nc.vector.scalar_tensor_tensor(c0[:, :F], atn[:, :F], -2.0,
                                               spi[:, :F], OP.mult, OP.add)
                nc.vector.scalar_tensor_tensor(c1[:, :F], den, 0.0,
                                               c0[:, :F], OP.is_lt, OP.mult)
                nc.vector.scalar_tensor_tensor(sd[:, :F], atn[:, :F], 0.0,
                                               c1[:, :F], OP.add, OP.add,
                                               accum_out=sacc[:, i:i + 1])

            nc.vector.memset(sacc[:], 0.0)
            nc.vector.memset(minda[:], 1.0)
            with tc.tile_pool(name="psum", bufs=1, space="PSUM") as ppool:
                for sg in range(0 if SKIP_P1 else NBLK // SUPER):
                    for j in range(SUPER):
                        pass_a(ppool, sg * SUPER + j, j)
                    tc.no_sync_barrier()
                    for j in range(SUPER):
                        pass_b(sg * SUPER + j, j)
                    tc.no_sync_barrier()

            # ------------- final: depth * inside, partition-reduce -------
            inside = cpool.tile([128, NBLK], fp32)
            depth = cpool.tile([128, NBLK], fp32)
            contrib = cpool.tile([128, NBLK], fp32)
            nc.vector.tensor_scalar(inside[:], sacc[:], HALF_PI, None,
                                    OP.is_gt)
            nc.scalar.activation(depth[:], minda[:], AF.Sqrt, bias=beps[:])
            nc.vector.tensor_tensor(contrib[:], depth[:], inside[:], OP.mult)

            with tc.tile_pool(name="psum2", bufs=1, space="PSUM") as p2:
                lpsum = p2.tile([NBLK, 1], fp32)
                nc.tensor.matmul(lpsum[:], contrib[:], ones[:])
                loss_sb = cpool.tile([NBLK, 1], fp32)
                nc.scalar.activation(loss_sb[:], lpsum[:], AF.Copy)
                nc.sync.dma_start(loss_d[:], loss_sb[:])


def _build():
    global _compiled
    if _compiled is not None:
        return _compiled
    import concourse.bacc as bacc
    import concourse.mybir as mybir
    import concourse.tile as tile

    nc = bacc.Bacc("TRN2", target_bir_lowering=False, debug=False,
                   num_devices=NCORES)
    fp32 = mybir.dt.float32
    pts_d = nc.dram_tensor("pts", (128, 2, NBD, 3), fp32, kind="ExternalInput").ap()
    faces_d = nc.dram_tensor("faces", (1, 2, 3, 512), fp32, kind="ExternalInput").ap()
    cst_d = nc.dram_tensor("cst", (128, 8), fp32, kind="ExternalInput").ap()
    extra_d = nc.dram_tensor("extra", (1, PPAD), fp32, kind="ExternalInput").ap()
    loss_d = nc.dram_tensor("loss", (NBLK, 1), fp32, kind="ExternalOutput").ap()

    with tile.TileContext(nc) as tc:
        _kernel_body(tc, pts_d, faces_d, cst_d, extra_d, loss_d)
    nc.compile()
    _compiled = nc
    return nc


# --------------------------------------------------------------------------
# cached jitted runner + entry point
# --------------------------------------------------------------------------

def _build_runner():
    global _runner
    if _runner is not None:
        return _runner
    import jax
    from jax.sharding import Mesh, PartitionSpec
    from jax.experimental.shard_map import shard_map
    import concourse.mybir as mybir
    from concourse.bass2jax import (_bass_exec_p, partition_id_tensor,
                                    install_neuronx_cc_hook)

    nc = _build()
    install_neuronx_cc_hook()
    pname = nc.partition_id_tensor.name if nc.partition_id_tensor else None
    in_names, out_names, out_avals, zero_outs = [], [], [], []
    for alloc in nc.m.functions[0].allocations:
        if not isinstance(alloc, mybir.MemoryLocationSet):
            continue
        name = alloc.memorylocations[0].name
        if alloc.kind == "ExternalInput":
            if name != pname:
                in_names.append(name)
        elif alloc.kind == "ExternalOutput":
            out_names.append(name)
            shape = tuple(alloc.tensor_shape)
            dtype = mybir.dt.np(alloc.dtype)
            out_avals.append(jax.core.ShapedArray(shape, dtype))
            zero_outs.append(np.zeros(shape, dtype))
    n_params, n_outs = len(in_names), len(out_avals)
    in_names_full = in_names + out_names + ([pname] if pname else [])

    def _body(*args):
        operands = list(args)
        if pname is not None:
            operands.append(partition_id_tensor())
        return tuple(_bass_exec_p.bind(
            *operands, out_avals=tuple(out_avals), in_names=tuple(in_names_full),
            out_names=tuple(out_names), lowering_input_output_aliases=(),
            sim_require_finite=True, sim_require_nnan=True, nc=nc))

    devices = jax.devices()[:NCORES]
    mesh = Mesh(np.asarray(devices), ("core",))
    in_specs = (PartitionSpec("core"),) * (n_params + n_outs)
    out_specs = (PartitionSpec("core"),) * len(out_names)
    sharded = jax.jit(
        shard_map(_body, mesh=mesh, in_specs=in_specs, out_specs=out_specs,
                  check_rep=False),
        donate_argnums=tuple(range(n_params, n_params + n_outs)),
        keep_unused=True)
    czero_shapes = [((NCORES * z.shape[0],) + z.shape[1:], z.dtype)
                    for z in zero_outs]
    _runner = (sharded, in_names, czero_shapes)
    return _runner


def kernel(**inputs) -> np.ndarray:
    global last_exec_time_ns
    pts, faces, cst, extra = _host_prep(inputs)
    sharded, in_names, czero_shapes = _build_runner()
    maps = _in_maps(pts, faces, cst, extra)
    concat_in = [np.concatenate([maps[c][nm] for c in range(NCORES)], axis=0)
                 for nm in in_names]
    zeros = [np.zeros(s, d) for s, d in czero_shapes]
    out = sharded(*concat_in, *zeros)
    last_exec_time_ns = None

    o0 = np.asarray(out[0]).reshape(NCORES, NBLK)
    loss = np.zeros(B, np.float32)
    for c in range(NCORES):
        # block i = (b_loc*2 + dir)*2 + chunk
        loss[c * NB:(c + 1) * NB] = o0[c].reshape(NB, 4).sum(axis=1)
    return loss


# revision 15
# speedup vs baseline: 9.8928x; 1.0273x over previous
"""Trainium2 Bass kernel for nn_HandIntersectionLoss.

Strategy
--------
Pure data parallel over batch: 64 batches -> 8 cores x 8 local batches.

Wall-clock per call is dominated by the axon tunnel, so the host ships
only the gathered hand points (~140KB/core) and the device derives all
per-(batch,face) matmul constants itself:

  phase 0 (device):
    - one-hot face matrices from f32 face indices (K=1 broadcast matmul
      + is_equal against shipped iota columns)
    - triangle corners A,B,C per (batch,dir) via 2-chunk accumulated
      gather matmuls:  corners[3,500] = pts[128,3]^T @ onehot[128,500]
    - edges E1=B-A, E2=C-A; normal n = E1 x E2 via permutation-matmul
      rotations (engines cannot read partition offsets != 0)
    - dots |A|^2,.., 2A.B,.., 2A.n via ones/twos-column reduce matmuls
    - constants assembled into a persistent `staged` SBUF tile
      ([65,7,512]: 4 rows per (batch,dir) + shared coefficient row)
      via SBUF->SBUF DMAs (the only legal cross-partition mover)

  phase 1 (device): the proven compute loop. Per 128-point block:
    K=5 matmuls against staged constants produce la^2,lb^2,lc^2,
    2ab,2bc,2ca, 2det for [128 points x 500 faces]; per-element chain
    (denominator + range-reduced atan2) on DVE/ACT:

      atan2(det, den) = 2*atan(det / (rho + |den|))            (den >= 0)
                      = sign(det)*pi - 2*atan(det/(rho+|den|)) (den < 0)
      rho = sqrt(det^2 + den^2 + 1e-20)

    inside(p) <=> sum_f half > pi/2.  Min-distance via the same matmul
    trick against derived vert constants (mrhs) + free-dim min-reduce.
    Scalar-engine table sets force the two-pass structure (sqrt vs
    arctan live in different ACT table sets), staged in super-groups.

The jitted shard_map callable is cached across kernel() calls so repeat
calls skip jax retrace/XLA recompile entirely.

Group semantics (raw, no halving on device):
  g0..2: xyz=A|B|C,       c3=|A|^2..,  w=1
  g3..5: xyz=(A+B)..raw,  c3=2A.B..,   w=2   -> col = 2*(A-p).(B-p)
  g6:    xyz=n raw,       c3=2*A.n,    w=0   -> col = 2*det
pass_a compensates with x0.5 folded into existing scalar_tensor_tensor.
"""
import os
import sys
import numpy as np

sys.path.insert(0, '/opt/trn_rl_repo')

B, V_FULL, V_HAND, V_LOOP, N_FACES = 64, 6890, 250, 20, 500
P = V_HAND + 1          # 251 points/verts per hand (incl. lid)
PPAD = 256
NCORES = 8
NB = B // NCORES        # local batches per core
NBD = NB * 2            # (batch, dir) pairs per core
NBLK = NBD * 2          # blocks per core: x2 point-chunks of 128
SUPER = 8               # blocks per two-pass super-group
F = N_FACES
HALF_PI = float(np.pi / 2)

_compiled = None
SKIP_P1 = False
_runner = None
last_exec_time_ns = None


# --------------------------------------------------------------------------
# host prep: index gathers only (all heavy constant math moved on-device)
# --------------------------------------------------------------------------

# preallocated per-call buffers (pad columns written once; concat layouts
# built directly to skip per-core copies)
_pts_host = np.full((B, 2, PPAD, 3), 1e2, np.float32)
_pts_concat = np.empty((NCORES * 128, 2, NBD, 3), np.float32)
_faces_concat = np.full((NCORES, 2, 3, 512), 300.0, np.float32)
_cst_concat = np.zeros((NCORES * 128, 8), np.float32)
for _c in range(NCORES):
    _cs = _cst_concat[_c * 128:(_c + 1) * 128]
    _cs[:, 0] = np.arange(128, dtype=np.float32)
    _cs[:, 1] = np.arange(128, 256, dtype=np.float32)
    for _m in range(3):
        _cs[(_m + 1) % 3, 2 + _m] = 1.0    # P1 (rot1)
        _cs[(_m + 2) % 3, 5 + _m] = 1.0    # P2 (rot2)
_extra_concat = np.ascontiguousarray(
    np.broadcast_to(np.arange(PPAD, dtype=np.float32), (NCORES, PPAD)))


def _host_prep(inputs):
    verts = np.asarray(inputs['verts_batch'], dtype=np.float32)
    hi = [np.asarray(inputs['hand_verts_inds_left']),
          np.asarray(inputs['hand_verts_inds_right'])]
    li = [np.asarray(inputs['hand_loop_verts_inds_left']),
          np.asarray(inputs['hand_loop_verts_inds_right'])]
    fc = [np.asarray(inputs['hand_faces_left']),
          np.asarray(inputs['hand_faces_right'])]

    # pad stays 1e2 from init (pad cols never overwritten)
    for d in range(2):
        _pts_host[:, d, :V_HAND] = verts[:, hi[d]]
        _pts_host[:, d, V_HAND] = verts[:, li[d]].mean(axis=1,
                                                       dtype=np.float32)

    # [core*128, 2kk, bd, 3] gather layout in one strided copy
    _pts_concat.reshape(NCORES, 128, 2, NBD, 3)[:] = \
        _pts_host.reshape(NCORES, NBD, 2, 128, 3).transpose(0, 3, 2, 1, 4)
    for s in range(2):
        _faces_concat[:, s, :, :F] = fc[s].T.astype(np.float32)[None]
    return [_pts_concat, _faces_concat, _cst_concat, _extra_concat]


# --------------------------------------------------------------------------
# device kernel
# --------------------------------------------------------------------------

def _kernel_body(tc, pts_d, faces_d, cst_d, extra_d, loss_d):
    import concourse.mybir as mybir
    nc = tc.nc
    fp32 = mybir.dt.float32
    AF = mybir.ActivationFunctionType
    OP = mybir.AluOpType
    AX = mybir.AxisListType.X

    with tc.tile_pool(name="const", bufs=1) as cpool:
        lhsT_sb = cpool.tile([5, NBD, PPAD], fp32)
        mrhs_sb = cpool.tile([5, NBD, PPAD], fp32)
        staged = cpool.tile([65, 7, 512], fp32)
        ones = cpool.tile([128, 1], fp32)
        beps = cpool.tile([128, 1], fp32)
        sacc = cpool.tile([128, NBLK], fp32)
        minda = cpool.tile([128, NBLK], fp32)
        nc.vector.memset(ones[:], 1.0)
        nc.vector.memset(beps[:], 1e-12)

        # ---------------- phase 0: derive constants on device ----------
        with tc.tile_pool(name="ph0", bufs=1) as zp:
            ones1 = zp.tile([1, 128], fp32)
            ones3 = zp.tile([3, 1], fp32)
            twos3 = zp.tile([3, 1], fp32)
            nc.vector.memset(ones1[:], 1.0)
            nc.vector.memset(ones3[:], 1.0)
            nc.vector.memset(twos3[:], 2.0)
            pts_sb = zp.tile([128, 2, NBD, 3], fp32)
            faces_sb = zp.tile([1, 2, 3, 512], fp32)
            cst_sb = zp.tile([128, 8], fp32)
            extra_sb = zp.tile([1, PPAD], fp32)
            nc.sync.dma_start(pts_sb[:], pts_d[:])
            nc.sync.dma_start(faces_sb[:], faces_d[:])
            nc.sync.dma_start(cst_sb[:], cst_d[:])
            nc.sync.dma_start(extra_sb[:], extra_d[:])

            # shared coefficient row -> staged[64]
            rc = zp.tile([1, 7, 512], fp32)
            nc.vector.memset(rc[:, 0:3, :], 1.0)
            nc.vector.memset(rc[:, 3:6, :], 2.0)
            nc.vector.memset(rc[:, 6:7, :], 0.0)
            nc.sync.dma_start(staged[64:65], rc[:])

            # one-hot face matrices per hand s, corner k, K-chunk kk
            # + identity one-hot (for pts transposition via gather matmul)
            oh = zp.tile([128, 2, 3, 2, 512], fp32)
            idh = zp.tile([128, 2, PPAD], fp32)
            PT = zp.tile([3, PPAD], fp32)
            SQ = zp.tile([3, PPAD], fp32)
            sqrow = zp.tile([1, NBD, PPAD], fp32)
            onesrow = zp.tile([1, NBD, PPAD], fp32)
            nc.vector.memset(onesrow[:], 1.0)
            with tc.tile_pool(name="ph0bc", bufs=1, space="PSUM") as bp:
                bc = bp.tile([128, 3, 512], fp32)
                bcid = bp.tile([128, PPAD], fp32)
                ptp = bp.tile([3, PPAD], fp32)
                sqp = bp.tile([1, PPAD], fp32)
                for s in range(2):
                    for k in range(3):
                        nc.tensor.matmul(bc[:, k, :], ones1[:],
                                         faces_sb[:, s, k, :])
                    for k in range(3):
                        for kk in range(2):
                            nc.vector.tensor_scalar(
                                oh[:, s, k, kk, :], bc[:, k, :],
                                cst_sb[:, kk:kk + 1], None, OP.is_equal)
                nc.tensor.matmul(bcid[:], ones1[:], extra_sb[:])
                for kk in range(2):
                    nc.vector.tensor_scalar(idh[:, kk, :], bcid[:],
                                            cst_sb[:, kk:kk + 1], None,
                                            OP.is_equal)
                # lhsT rows from pts: -2*pts^T via identity-gather matmuls,
                # |p|^2 via square + ones3-reduce
                for bd in range(NBD):
                    nc.tensor.matmul(ptp[:], pts_sb[:, 0, bd, :],
                                     idh[:, 0, :], start=True, stop=False)
                    nc.tensor.matmul(ptp[:], pts_sb[:, 1, bd, :],
                                     idh[:, 1, :], start=False, stop=True)
                    nc.scalar.mul(lhsT_sb[0:3, bd, :], ptp[:], -2.0)
                    nc.scalar.activation(PT[:], ptp[:], AF.Copy)
                    nc.vector.tensor_tensor(SQ[:], PT[:], PT[:], OP.mult)
                    nc.tensor.matmul(sqp[:], ones3[:], SQ[:])
                    nc.scalar.activation(sqrow[:, bd, :], sqp[:], AF.Copy)
            nc.sync.dma_start(lhsT_sb[3:4], onesrow[:])
            nc.sync.dma_start(lhsT_sb[4:5], sqrow[:])

            # mrhs: rows0..2 = -0.5*lhsT rows0..2 (= vert xyz),
            # row3 <- lhsT row4 (|v|^2), row4 <- lhsT row3 (ones)
            nc.vector.tensor_scalar(mrhs_sb[0:3], lhsT_sb[0:3], -0.5, None,
                                    OP.mult)
            nc.sync.dma_start(mrhs_sb[3:4], lhsT_sb[4:5])
            nc.sync.dma_start(mrhs_sb[4:5], lhsT_sb[3:4])

            Ac = zp.tile([3, 512], fp32)
            Bc = zp.tile([3, 512], fp32)
            Cc = zp.tile([3, 512], fp32)
            E1 = zp.tile([3, 512], fp32)
            E2 = zp.tile([3, 512], fp32)
            rotc = zp.tile([3, 4, 512], fp32)
            t1 = zp.tile([3, 512], fp32)
            t2 = zp.tile([3, 512], fp32)
            NN = zp.tile([3, 512], fp32)
            PRD = zp.tile([3, 7, 512], fp32)
            MID = zp.tile([3, 3, 512], fp32)
            C3r = zp.tile([1, 7, 512], fp32)

            with tc.tile_pool(name="ph0ps", bufs=1, space="PSUM") as pp0:
                crn = [pp0.tile([3, 512], fp32, name=f"crn{t}", tag=t)
                       for t in "abc"]
                rot = pp0.tile([3, 4, 512], fp32)
                c3p = pp0.tile([1, 512], fp32)
                for bd in range(NBD):
                    other = bd ^ 1
                    s = other & 1
                    for k in range(3):
                        nc.tensor.matmul(crn[k][:], pts_sb[:, 0, other, :],
                                         oh[:, s, k, 0, :],
                                         start=True, stop=False)
                        nc.tensor.matmul(crn[k][:], pts_sb[:, 1, other, :],
                                         oh[:, s, k, 1, :],
                                         start=False, stop=True)
                    nc.scalar.activation(Ac[:], crn[0][:], AF.Copy)
                    nc.scalar.activation(Bc[:], crn[1][:], AF.Copy)
                    nc.scalar.activation(Cc[:], crn[2][:], AF.Copy)
                    nc.vector.tensor_tensor(E1[:], Bc[:], Ac[:], OP.subtract)
                    nc.vector.tensor_tensor(E2[:], Cc[:], Ac[:], OP.subtract)
                    # n = E1 x E2 via rotations: rot1/rot2 = P1^T/P2^T
                    nc.tensor.matmul(rot[:, 0, :], cst_sb[0:3, 2:5], E1[:])
                    nc.tensor.matmul(rot[:, 1, :], cst_sb[0:3, 5:8], E2[:])
                    nc.tensor.matmul(rot[:, 2, :], cst_sb[0:3, 5:8], E1[:])
                    nc.tensor.matmul(rot[:, 3, :], cst_sb[0:3, 2:5], E2[:])
                    nc.scalar.activation(rotc[:], rot[:], AF.Copy)
                    nc.vector.tensor_tensor(t1[:], rotc[:, 0, :],
                                            rotc[:, 1, :], OP.mult)
                    nc.vector.tensor_tensor(t2[:], rotc[:, 2, :],
                                            rotc[:, 3, :], OP.mult)
                    nc.vector.tensor_tensor(NN[:], t1[:], t2[:], OP.subtract)
                    # products for the c3 reduces
                    nc.vector.tensor_tensor(PRD[:, 0, :], Ac[:], Ac[:], OP.mult)
                    nc.vector.tensor_tensor(PRD[:, 1, :], Bc[:], Bc[:], OP.mult)
                    nc.vector.tensor_tensor(PRD[:, 2, :], Cc[:], Cc[:], OP.mult)
                    nc.vector.tensor_tensor(PRD[:, 3, :], Ac[:], Bc[:], OP.mult)
                    nc.vector.tensor_tensor(PRD[:, 4, :], Bc[:], Cc[:], OP.mult)
                    nc.vector.tensor_tensor(PRD[:, 5, :], Cc[:], Ac[:], OP.mult)
                    nc.vector.tensor_tensor(PRD[:, 6, :], Ac[:], NN[:], OP.mult)
                    nc.vector.tensor_tensor(MID[:, 0, :], Ac[:], Bc[:], OP.add)
                    nc.vector.tensor_tensor(MID[:, 1, :], Bc[:], Cc[:], OP.add)
                    nc.vector.tensor_tensor(MID[:, 2, :], Cc[:], Ac[:], OP.add)
                    for g in range(7):
                        nc.tensor.matmul(c3p[:], ones3[:] if g < 3 else twos3[:],
                                         PRD[:, g, :])
                        nc.scalar.activation(C3r[:, g, :], c3p[:], AF.Copy)
                    # assemble staged rows via SBUF->SBUF DMA
                    r0 = staged[4 * bd:4 * bd + 3]
                    nc.sync.dma_start(r0[:, 0, :], Ac[:])
                    nc.sync.dma_start(r0[:, 1, :], Bc[:])
                    nc.sync.dma_start(r0[:, 2, :], Cc[:])
                    nc.sync.dma_start(r0[:, 3:6, :], MID[:])
                    nc.sync.dma_start(r0[:, 6, :], NN[:])
                    nc.sync.dma_start(staged[4 * bd + 3:4 * bd + 4], C3r[:])

        # ---------------- phase 1: main compute loop --------------------
        with (
            tc.tile_pool(name="store", bufs=1) as spool,
            tc.tile_pool(name="stage", bufs=2) as stpool,
            tc.tile_pool(name="iface", bufs=2) as ipool,
            tc.tile_pool(name="dve", bufs=1) as vpool,
        ):
            denoms = spool.tile([128, SUPER, 512], fp32)
            tts = spool.tile([128, SUPER, 512], fp32)

            def pass_a(ppool, i, j):
                bd, ch = divmod(i, 2)
                if ch == 0:
                    fstage = stpool.tile([5, 7, 512], fp32, tag="fstage")
                    nc.sync.dma_start(fstage[0:4], staged[4 * bd:4 * bd + 4])
                    nc.sync.dma_start(fstage[4:5], staged[64:65])
                    pass_a.stage = fstage
                fstage = pass_a.stage
                lhs = lhsT_sb[:, bd, ch * 128:(ch + 1) * 128]

                wind = ppool.tile([128, 7, 512], fp32, tag="wind")
                md = ppool.tile([128, 256], fp32, tag="md")

                for g in range(7):
                    nc.tensor.matmul(wind[:, g, :F], lhs, fstage[:, g, :F])
                nc.tensor.matmul(md[:, :P], lhs, mrhs_sb[:, bd ^ 1, :P])

                # min-distance: free-dim min, clamp at 0 (matmul roundoff)
                mind = vpool.tile([128, 1], fp32, tag="mind")
                nc.vector.tensor_reduce(mind[:], md[:, :P], AX, OP.min)
                nc.vector.tensor_scalar(minda[:, i:i + 1], mind[:], 0.0, None,
                                        OP.max)

                # norms: clamp squared lengths at 0, sqrt
                rl = ipool.tile([128, 3, 512], fp32, tag="rl")
                for g in range(3):
                    nc.scalar.activation(rl[:, g, :F], wind[:, g, :F], AF.Relu)
                la = ipool.tile([128, 512], fp32, tag="la")
                lb = ipool.tile([128, 512], fp32, tag="lb")
                lc = ipool.tile([128, 512], fp32, tag="lc")
                nc.scalar.activation(la[:, :F], rl[:, 0, :F], AF.Sqrt)
                nc.scalar.activation(lb[:, :F], rl[:, 1, :F], AF.Sqrt)
                nc.scalar.activation(lc[:, :F], rl[:, 2, :F], AF.Sqrt)
                dets = ipool.tile([128, 512], fp32, tag="dets")
                nc.scalar.mul(dets[:, :F], wind[:, 6, :F], 0.5)

                # denominator chain; wind groups 3..5 hold 2ab/2bc/2ca so
                # fold the x0.5 into the scalar_tensor_tensor ops
                u = vpool.tile([128, 512], fp32, tag="u")
                r4 = vpool.tile([128, 512], fp32, tag="r4")
                s5 = vpool.tile([128, 512], fp32, tag="s5")
                v = vpool.tile([128, 512], fp32, tag="v")
                w = vpool.tile([128, 512], fp32, tag="w")
                t6 = vpool.tile([128, 512], fp32, tag="t6")
                nc.vector.scalar_tensor_tensor(r4[:, :F], wind[:, 4, :F], 0.5,
                                               la[:, :F], OP.mult, OP.mult)
                nc.vector.scalar_tensor_tensor(s5[:, :F], wind[:, 5, :F], 0.5,
                                               lb[:, :F], OP.mult, OP.mult)
                nc.vector.tensor_tensor(u[:, :F], la[:, :F], lb[:, :F], OP.mult)
                nc.vector.scalar_tensor_tensor(v[:, :F], wind[:, 3, :F], 0.5,
                                               u[:, :F], OP.mult, OP.add)

                w_ = w[:, :F]
                nc.vector.tensor_tensor(w_, v[:, :F], lc[:, :F], OP.mult)
                nc.vector.tensor_tensor(t6[:, :F], r4[:, :F], s5[:, :F], OP.add)
                den = denoms[:, j, :F]
                nc.vector.tensor_tensor(den, w_, t6[:, :F], OP.add)

                # half-angle atan2 range reduction: tt = det / (rho + |den|)
                xx = ipool.tile([128, 512], fp32, tag="xx")
                yy = ipool.tile([128, 512], fp32, tag="yy")
                ss = vpool.tile([128, 512], fp32, tag="ss", bufs=2)
                rho = ipool.tile([128, 512], fp32, tag="rho")
                axd = ipool.tile([128, 512], fp32, tag="axd")
                dd = vpool.tile([128, 512], fp32, tag="dd")
                rd = vpool.tile([128, 512], fp32, tag="rd")
                nc.scalar.activation(xx[:, :F], den, AF.Square)
                nc.scalar.activation(yy[:, :F], dets[:, :F], AF.Square)
                nc.vector.scalar_tensor_tensor(ss[:, :F], xx[:, :F], 1e-20,
                                               yy[:, :F], OP.add, OP.add)
                nc.scalar.activation(rho[:, :F], ss[:, :F], AF.Sqrt)
                nc.scalar.activation(axd[:, :F], den, AF.Abs)
                nc.vector.tensor_tensor(dd[:, :F], rho[:, :F], axd[:, :F],
                                        OP.add)
                nc.vector.reciprocal_approx_fast(rd[:, :F], dd[:, :F])
                nc.vector.tensor_tensor(tts[:, j, :F], dets[:, :F], rd[:, :F],
                                        OP.mult)

            def pass_b(i, j):
                den = denoms[:, j, :F]
                tt = tts[:, j, :F]
                sgn = ipool.tile([128, 512], fp32, tag="sgn")
                spi = ipool.tile([128, 512], fp32, tag="spi")
                atn = ipool.tile([128, 512], fp32, tag="atn")
                c0 = vpool.tile([128, 512], fp32, tag="c0")
                c1 = vpool.tile([128, 512], fp32, tag="c1")
                sd = vpool.tile([128, 512], fp32, tag="sd")
                nc.scalar.activation(sgn[:, :F], tt, AF.Sign)
                nc.scalar.mul(spi[:, :F], sgn[:, :F], HALF_PI)
                nc.scalar.activation(atn[:, :F], tt, AF.Arctan)
                # half = atn + [den<0]*(pi/2*sign(det) - 2*atn)
                nc.vector.scalar_tensor_tensor(c0[:, :F], atn[:, :F], -2.0,
                                               spi[:, :F], OP.mult, OP.add)
                nc.vector.scalar_tensor_tensor(c1[:, :F], den, 0.0,
                                               c0[:, :F], OP.is_lt, OP.mult)
                nc.vector.scalar_tensor_tensor(sd[:, :F], atn[:, :F], 0.0,
                                               c1[:, :F], OP.add, OP.add,
                                               accum_out=sacc[:, i:i + 1])

            nc.vector.memset(sacc[:], 0.0)
            nc.vector.memset(minda[:], 1.0)
            with tc.tile_pool(name="psum", bufs=1, space="PSUM") as ppool:
                for sg in range(0 if SKIP_P1 else NBLK // SUPER):
                    for j in range(SUPER):
                        pass_a(ppool, sg * SUPER + j, j)
                    tc.no_sync_barrier()
                    for j in range(SUPER):
                        pass_b(sg * SUPER + j, j)
                    tc.no_sync_barrier()

            # ------------- final: depth * inside, partition-reduce -------
            inside = cpool.tile([128, NBLK], fp32)
            depth = cpool.tile([128, NBLK], fp32)
            contrib = cpool.tile([128, NBLK], fp32)
            nc.vector.tensor_scalar(inside[:], sacc[:], HALF_PI, None,
                                    OP.is_gt)
            nc.scalar.activation(depth[:], minda[:], AF.Sqrt, bias=beps[:])
            nc.vector.tensor_tensor(contrib[:], depth[:], inside[:], OP.mult)

            with tc.tile_pool(name="psum2", bufs=1, space="PSUM") as p2:
                lpsum = p2.tile([NBLK, 1], fp32)
                nc.tensor.matmul(lpsum[:], contrib[:], ones[:])
                loss_sb = cpool.tile([NBLK, 1], fp32)
                nc.scalar.activation(loss_sb[:], lpsum[:], AF.Copy)
                nc.sync.dma_start(loss_d[:], loss_sb[:])


def _build():
    global _compiled
    if _compiled is not None:
        return _compiled
    import concourse.bacc as bacc
    import concourse.mybir as mybir
    import concourse.tile as tile

    nc = bacc.Bacc("TRN2", target_bir_lowering=False, debug=False,
                   num_devices=NCORES)
    fp32 = mybir.dt.float32
    pts_d = nc.dram_tensor("pts", (128, 2, NBD, 3), fp32, kind="ExternalInput").ap()
    faces_d = nc.dram_tensor("faces", (1, 2, 3, 512), fp32, kind="ExternalInput").ap()
    cst_d = nc.dram_tensor("cst", (128, 8), fp32, kind="ExternalInput").ap()
    extra_d = nc.dram_tensor("extra", (1, PPAD), fp32, kind="ExternalInput").ap()
    loss_d = nc.dram_tensor("loss", (NBLK, 1), fp32, kind="ExternalOutput").ap()

    with tile.TileContext(nc) as tc:
        _kernel_body(tc, pts_d, faces_d, cst_d, extra_d, loss_d)
    nc.compile()
    _compiled = nc
    return nc


# --------------------------------------------------------------------------
# cached jitted runner + entry point
# --------------------------------------------------------------------------

def _build_runner():
    global _runner
    if _runner is not None:
        return _runner
    import jax
    from jax.sharding import Mesh, PartitionSpec
    from jax.experimental.shard_map import shard_map
    import concourse.mybir as mybir
    from concourse.bass2jax import (_bass_exec_p, partition_id_tensor,
                                    install_neuronx_cc_hook)

    nc = _build()
    install_neuronx_cc_hook()
    pname = nc.partition_id_tensor.name if nc.partition_id_tensor else None
    in_names, out_names, out_avals, zero_outs = [], [], [], []
    for alloc in nc.m.functions[0].allocations:
        if not isinstance(alloc, mybir.MemoryLocationSet):
            continue
        name = alloc.memorylocations[0].name
        if alloc.kind == "ExternalInput":
            if name != pname:
                in_names.append(name)
        elif alloc.kind == "ExternalOutput":
            out_names.append(name)
            shape = tuple(alloc.tensor_shape)
            dtype = mybir.dt.np(alloc.dtype)
            out_avals.append(jax.core.ShapedArray(shape, dtype))
            zero_outs.append(np.zeros(shape, dtype))
    n_params, n_outs = len(in_names), len(out_avals)
    in_names_full = in_names + out_names + ([pname] if pname else [])

    def _body(*args):
        operands = list(args)
        if pname is not None:
            operands.append(partition_id_tensor())
        return tuple(_bass_exec_p.bind(
            *operands, out_avals=tuple(out_avals), in_names=tuple(in_names_full),
            out_names=tuple(out_names), lowering_input_output_aliases=(),
            sim_require_finite=True, sim_require_nnan=True, nc=nc))

    devices = jax.devices()[:NCORES]
    mesh = Mesh(np.asarray(devices), ("core",))
    in_specs = (PartitionSpec("core"),) * (n_params + n_outs)
    out_specs = (PartitionSpec("core"),) * len(out_names)
    sharded = jax.jit(
        shard_map(_body, mesh=mesh, in_specs=in_specs, out_specs=out_specs,
                  check_rep=False),
        donate_argnums=tuple(range(n_params, n_params + n_outs)),
        keep_unused=True)
    czero_shapes = [((NCORES * z.shape[0],) + z.shape[1:], z.dtype)
                    for z in zero_outs]
    _runner = (sharded, in_names, czero_shapes)
    return _runner


def kernel(**inputs) -> np.ndarray:
    global last_exec_time_ns
    by_name = dict(zip(["pts", "faces", "cst", "extra"], _host_prep(inputs)))
    sharded, in_names, czero_shapes = _build_runner()
    concat_in = [by_name[nm] for nm in in_names]
    zeros = [np.zeros(s, d) for s, d in czero_shapes]
    out = sharded(*concat_in, *zeros)
    last_exec_time_ns = None

    o0 = np.asarray(out[0]).reshape(NCORES, NBLK)
    loss = np.zeros(B, np.float32)
    for c in range(NCORES):
        # block i = (b_loc*2 + dir)*2 + chunk
        loss[c * NB:(c + 1) * NB] = o0[c].reshape(NB, 4).sum(axis=1)
    return loss
